# revision 22
# baseline (speedup 1.0000x reference)
"""Fused decoder-layer kernel for Trainium2 (8 NeuronCores, data-parallel over batch).

Self-contained: hardcodes shapes B=64, S=128, D=1024, H=8, DK=128, DFF=2048.

Strategy:
  - Shard batch 8-ways (8 batches = 1024 tokens per core). No collectives.
  - Activations kept feature-major ("transposed", [feat, tok]) on device so
    every matmul Y = X @ W.T becomes Y.T with the contraction dim on SBUF
    partitions; weights are pre-transposed on host and cast bf16.
  - The layer is processed in two token-halves of 512 (4 batches each);
    attention and layernorm are token-local, so the halves are independent
    pipelines that share SBUF slots (Tile inserts the waits).
  - Attention without transposes: scoresT[k,q] = K_h^T-free matmul, exp on
    ScalarE (shift-free softmax: logits are O(1) for this module), both
    V-matmuls use token-major V as the stationary operand, and the softmax
    normalization (per-q column scale) is applied at the final drain using a
    GpSimd partition-all-reduce of exp(scores). One PSUM bank per unit
    (scores/av/ctx packed as column slices) keeps many units in flight.
  - LayerNorm over features (= partitions) via PE ones-reduction for
    sum / sum-of-squares, GpSimd partition_broadcast for mean/invstd rows.
  - Residuals and layernorm in fp32; matmul operands bf16 (fp32 PSUM accum).
  - Streamed FFN weights are DMA'd from the GpSimd DGE (the Sync DGE is a
    serial ~0.6us-per-descriptor bottleneck); everything else from Sync.
"""

import sys

if "/opt/trn_rl_repo" not in sys.path:
    sys.path.insert(0, "/opt/trn_rl_repo")

import numpy as np
import ml_dtypes

B, S, D, H = 64, 128, 1024, 8
DK = D // H
DFF = 2048
D4 = 4 * D
EPS = 1e-12
NCORES = 8
BPC = B // NCORES          # batches per core
TOK = BPC * S              # tokens per core = 1024
TH = 512                   # token half processed per pipeline pass
NTH = TOK // TH            # 2
BPT = TH // S              # batches per half = 4
C = 128                    # chunk / partition size
KD = D // C                # 8
KFF = DFF // C             # 16
K4 = D4 // C               # 32
SCALE = 1.0 / np.sqrt(DK)

# packed bias/ln columns: name -> (start, count)
_BCOLS = {}
_off = 0
for _nm, _cnt in [("bqs", KD), ("bk", KD), ("bo", KD), ("b1", KFF),
                  ("b2", K4), ("b3", KD), ("l1w", KD), ("l1b", KD),
                  ("l2w", KD), ("l2b", KD)]:
    _BCOLS[_nm] = (_off, _cnt)
    _off += _cnt
NBCOL = _off

_nc_cache = {}
_trace = {"on": False, "res": None}


def _build():
    import concourse.bass as bass
    import concourse.mybir as mybir
    import concourse.tile as tile
    import concourse.bass_isa as bass_isa
    from concourse import bacc

    f32 = mybir.dt.float32
    b16 = mybir.dt.bfloat16
    Alu = mybir.AluOpType
    Act = mybir.ActivationFunctionType
    AX = mybir.AxisListType

    nc = bacc.Bacc("TRN2", target_bir_lowering=False, debug=False,
                   num_devices=NCORES)

    # ---- DRAM I/O ------------------------------------------------------
    xT = nc.dram_tensor("xT", [D, TOK], f32, kind="ExternalInput")
    xTb = nc.dram_tensor("xTb", [D, TOK], b16, kind="ExternalInput")
    wqT = nc.dram_tensor("wqT", [D, D], b16, kind="ExternalInput")
    wkT = nc.dram_tensor("wkT", [D, D], b16, kind="ExternalInput")
    wvT = nc.dram_tensor("wvT", [D, D], b16, kind="ExternalInput")
    woT = nc.dram_tensor("woT", [D, D], b16, kind="ExternalInput")
    w1T = nc.dram_tensor("w1T", [D, DFF], b16, kind="ExternalInput")
    w2T = nc.dram_tensor("w2T", [DFF, D4], b16, kind="ExternalInput")
    w3T = nc.dram_tensor("w3T", [D4, D], b16, kind="ExternalInput")
    bcols = nc.dram_tensor("bcols", [C, NBCOL], f32, kind="ExternalInput")
    bv_ = nc.dram_tensor("bv_", [1, D], f32, kind="ExternalInput")
    outT = nc.dram_tensor("outT", [D, TOK], f32, kind="ExternalOutput")

    with tile.TileContext(nc) as tc:
        P = tc.alloc_tile_pool(name="main", bufs=1)
        psum = tc.alloc_tile_pool(name="psum", bufs=8, space="PSUM")

        def mk(shape, dtype, tag):
            return P.tile(shape, dtype, tag=tag, name=tag)

        ones = mk([C, 1], b16, "ones")
        nc.vector.memset(ones, 1.0)
        eps1 = mk([1, 1], f32, "eps1")
        nc.vector.memset(eps1, EPS)

        sb_bc = mk([C, NBCOL], f32, "bcols")
        nc.sync.dma_start(out=sb_bc, in_=bcols.ap())

        def col(name, c):
            s, n = _BCOLS[name]
            assert c < n
            return sb_bc[:, s + c:s + c + 1]

        sb_bvb = []
        for oh in range(NTH):
            t = mk([C, TH], f32, f"bvb{oh}")
            src = bv_.ap()[0:1, oh * TH:(oh + 1) * TH]
            nc.sync.dma_start(out=t, in_=src.partition_broadcast(C))
            sb_bvb.append(t)

        # ================= per-token-half pipeline =================
        for th in range(NTH):
            tsl = slice(th * TH, (th + 1) * TH)

            # --- load x (bf16 for matmuls, fp32 for residual) ---
            # --- V = x @ Wv.T + bv (token-major [tok, feat]) ---
            xb, wv_t = [], []
            for k in range(KD):
                t = mk([C, TH], b16, f"xb{k}")
                nc.sync.dma_start(out=t, in_=xTb.ap()[k * C:(k + 1) * C, tsl])
                xb.append(t)
                t = mk([C, D], b16, f"swA{k}")
                nc.sync.dma_start(out=t, in_=wvT.ap()[k * C:(k + 1) * C, :])
                wv_t.append(t)
            sv = [[None] * NTH for _ in range(BPT)]
            for tb in range(BPT):
                for oh in range(NTH):
                    pv = psum.tile([C, TH], f32, tag="ps",
                                   name=f"psv{tb}{oh}{th}")
                    for k in range(KD):
                        nc.tensor.matmul(
                            pv, xb[k][:, tb * C:(tb + 1) * C],
                            wv_t[k][:, oh * TH:(oh + 1) * TH],
                            start=(k == 0), stop=(k == KD - 1))
                    t = mk([C, TH], b16, f"sv{tb}_{oh}")
                    nc.vector.tensor_add(t, pv, sb_bvb[oh])
                    sv[tb][oh] = t

            # --- Q, K for all heads (feature-major [dk, tok]) ---
            wq_t, wk_t = [], []
            for k in range(KD):
                t = mk([C, D], b16, f"swA{8 + k}")
                nc.sync.dma_start(out=t, in_=wqT.ap()[k * C:(k + 1) * C, :])
                wq_t.append(t)
            for k in range(KD):
                t = mk([C, D], b16, f"swA{k}")  # reuse wv slots
                nc.sync.dma_start(out=t, in_=wkT.ap()[k * C:(k + 1) * C, :])
                wk_t.append(t)
            qh, kh = [None] * H, [None] * H
            for h in range(H):
                pq = psum.tile([C, TH], f32, tag="ps", name=f"psq{h}{th}")
                for k in range(KD):
                    nc.tensor.matmul(pq, wq_t[k][:, h * C:(h + 1) * C],
                                     xb[k], start=(k == 0), stop=(k == KD - 1))
                qh[h] = mk([C, TH], b16, f"q{h}")
                nc.scalar.activation(qh[h], pq, Act.Identity,
                                     bias=col("bqs", h), scale=SCALE)
                pk = psum.tile([C, TH], f32, tag="ps", name=f"psk{h}{th}")
                for k in range(KD):
                    nc.tensor.matmul(pk, wk_t[k][:, h * C:(h + 1) * C],
                                     xb[k], start=(k == 0), stop=(k == KD - 1))
                kh[h] = mk([C, TH], b16, f"k{h}")
                nc.scalar.activation(kh[h], pk, Act.Identity,
                                     bias=col("bk", h))

            # --- attention units (shift-free softmax, no transposes) ---
            ctx = [None] * KD
            for h in range(KD):
                ctx[h] = mk([C, TH], b16, f"ctx{h}")
            uidx = 0
            for h in range(H):
                rrow = mk([1, TH], f32, f"rro{h % 4}")
                cxu = mk([C, TH], b16, f"ctxu{h % 4}")
                for tb in range(BPT):
                    u = uidx % 6
                    uidx += 1
                    bsl = slice(tb * C, (tb + 1) * C)
                    vsl = sv[tb][h // BPT][:, (h % BPT) * C:(h % BPT + 1) * C]
                    pb = psum.tile([C, 4 * C], f32, tag="ps",
                                   name=f"pat{h}{tb}{th}")
                    # scoresT[k,q] (pre-scaled via Q)
                    nc.tensor.matmul(pb[:, 0:C], kh[h][:, bsl], qh[h][:, bsl],
                                     start=True, stop=True)
                    e = mk([C, C], b16, f"e{u}")
                    nc.scalar.activation(e, pb[:, 0:C], Act.Exp)
                    # column sums via PE ones-reduce into the 4th quarter
                    nc.tensor.matmul(pb[0:1, 3 * C:4 * C], ones, e,
                                     start=True, stop=True)
                    nc.vector.reciprocal(rrow[0:1, bsl],
                                         pb[0:1, 3 * C:4 * C])
                    # avT_unnorm = V^T-free matmul with E
                    nc.tensor.matmul(pb[:, C:2 * C], vsl, e,
                                     start=True, stop=True)
                    avb = mk([C, C], b16, f"avb{u}")
                    nc.scalar.copy(avb, pb[:, C:2 * C])
                    # ctxT_unnorm; normalization deferred to per-head pass
                    nc.tensor.matmul(pb[:, 2 * C:3 * C], vsl, avb,
                                     start=True, stop=True)
                    nc.scalar.copy(cxu[:, bsl], pb[:, 2 * C:3 * C])
                # normalize the head's ctx columns by 1/rowsum
                rall = mk([C, TH], f32, f"rall{h % 4}")
                nc.gpsimd.partition_broadcast(rall, rrow)
                nc.vector.tensor_mul(ctx[h], cxu, rall)

            # --- attn_out = ctx @ Wo.T + bo ; r1 = x + attn_out ---
            wo_t = []
            for k in range(KD):
                t = mk([C, D], b16, f"swA{8 + k}")  # reuse wq slots
                nc.sync.dma_start(out=t, in_=woT.ap()[k * C:(k + 1) * C, :])
                wo_t.append(t)
            xf = []
            for k in range(KD):
                t = mk([C, TH], f32, f"xf{k}")
                nc.sync.dma_start(out=t, in_=xT.ap()[k * C:(k + 1) * C, tsl])
                xf.append(t)
            r1 = [None] * KD
            for c in range(KD):
                po = psum.tile([C, TH], f32, tag="ps", name=f"pso{c}{th}")
                for k in range(KD):
                    nc.tensor.matmul(po, wo_t[k][:, c * C:(c + 1) * C],
                                     ctx[k], start=(k == 0), stop=(k == KD - 1))
                t = mk([C, TH], f32, f"r1h{c}")
                nc.vector.scalar_tensor_tensor(t, po, col("bo", c), xf[c],
                                               op0=Alu.add, op1=Alu.add)
                r1[c] = t

            # --- layernorm helper (over features = partitions) ---
            def layer_norm(src, wname, bname, out_mk, also_b16):
                vb, sq = [], []
                for c in range(KD):
                    tvb = mk([C, TH], b16, f"svb{c}")
                    nc.vector.tensor_copy(tvb, src[c])
                    vb.append(tvb)
                    tsq = mk([C, TH], b16, f"ssq{c}")
                    nc.scalar.activation(tsq, src[c], Act.Square)
                    sq.append(tsq)
                ps1 = psum.tile([1, TH], f32, tag="ps", name=f"ps1{th}")
                for c in range(KD):
                    nc.tensor.matmul(ps1, ones, vb[c], start=(c == 0),
                                     stop=(c == KD - 1))
                ps2 = psum.tile([1, TH], f32, tag="ps", name=f"ps2{th}")
                for c in range(KD):
                    nc.tensor.matmul(ps2, ones, sq[c], start=(c == 0),
                                     stop=(c == KD - 1))
                mu = mk([1, TH], f32, "rowA")
                nc.scalar.mul(mu, ps1, 1.0 / D)
                msq = mk([1, TH], f32, "rowB")
                nc.scalar.mul(msq, ps2, 1.0 / D)
                mu2 = mk([1, TH], f32, "rowC")
                nc.vector.tensor_mul(mu2, mu, mu)
                var = mk([1, TH], f32, "rowD")
                nc.vector.tensor_sub(var, msq, mu2)
                sd = mk([1, TH], f32, "rowC")
                nc.scalar.activation(sd, var, Act.Sqrt, bias=eps1)
                rinv = mk([1, TH], f32, "rowB")
                nc.vector.reciprocal(rinv, sd)
                mub = mk([C, TH], f32, "mub")
                nc.gpsimd.partition_broadcast(mub, mu)
                rb = mk([C, TH], f32, "rb")
                nc.gpsimd.partition_broadcast(rb, rinv)
                outs_f, outs_b = [], []
                for c in range(KD):
                    t1 = mk([C, TH], f32, "t1")
                    nc.vector.tensor_sub(t1, src[c], mub)
                    t2 = mk([C, TH], f32, f"t2{c % 2}")
                    nc.vector.tensor_mul(t2, t1, rb)
                    to = out_mk(c)
                    nc.scalar.activation(to, t2, Act.Identity,
                                         bias=col(bname, c),
                                         scale=col(wname, c))
                    outs_f.append(to)
                    if also_b16:
                        tb16 = mk([C, TH], b16, f"sv{c // 2}_{c % 2}")
                        nc.vector.tensor_copy(tb16, to)
                        outs_b.append(tb16)
                return outs_f, outs_b

            # --- LN1: h fp32 (r1 slots) + bf16 copy (sv slots) ---
            hf, hb = layer_norm(r1, "l1w", "l1b",
                                lambda c: mk([C, TH], f32, f"r1h{c}"), True)

            # --- f1 = relu(h @ W1.T + b1); W1 streamed via GpSimd DGE ---
            f1tag = [f"ctx{i}" for i in range(8)] + [f"q{i}" for i in range(8)]
            f1 = [None] * KFF
            for blk in range(KFF // 4):
                pf = [psum.tile([C, TH], f32, tag="ps",
                                name=f"psf1{blk}{i}{th}") for i in range(4)]
                for k in range(KD):
                    ws = mk([C, 4 * C], b16, f"ws{(blk * KD + k) % 12}")
                    eng = nc.gpsimd if k % 2 == 0 else nc.scalar
                    eng.dma_start(
                        out=ws, in_=w1T.ap()[k * C:(k + 1) * C,
                                             blk * 4 * C:(blk + 1) * 4 * C])
                    for i in range(4):
                        nc.tensor.matmul(pf[i], ws[:, i * C:(i + 1) * C],
                                         hb[k], start=(k == 0),
                                         stop=(k == KD - 1))
                for i in range(4):
                    c = blk * 4 + i
                    t = mk([C, TH], b16, f1tag[c])
                    nc.scalar.activation(t, pf[i], Act.Relu, bias=col("b1", c))
                    f1[c] = t

            # --- f2 = relu(f1 @ W2.T + b2) ---
            f2tag = [f"xf{i}" for i in range(8)] + \
                    [f"xb{i}" for i in range(8)] + \
                    [f"sv{i // 2}_{i % 2}" for i in range(8)] + \
                    [f"k{i}" for i in range(8)]
            f2 = [None] * K4
            for blk in range(K4 // 4):
                pf = [psum.tile([C, TH], f32, tag="ps",
                                name=f"psf2{blk}{i}{th}") for i in range(4)]
                for k in range(KFF):
                    ws = mk([C, 4 * C], b16, f"ws{(blk * KFF + k) % 12}")
                    eng = nc.gpsimd if k % 2 == 0 else nc.scalar
                    eng.dma_start(
                        out=ws, in_=w2T.ap()[k * C:(k + 1) * C,
                                             blk * 4 * C:(blk + 1) * 4 * C])
                    for i in range(4):
                        nc.tensor.matmul(pf[i], ws[:, i * C:(i + 1) * C],
                                         f1[k], start=(k == 0),
                                         stop=(k == KFF - 1))
                for i in range(4):
                    c = blk * 4 + i
                    t = mk([C, TH], b16, f2tag[c])
                    nc.scalar.activation(t, pf[i], Act.Relu, bias=col("b2", c))
                    f2[c] = t

            # --- f3 = f2 @ W3.T + b3 ; r2 = h + f3 ---
            r2 = [None] * KD
            for blk in range(KD // 4):
                pf = [psum.tile([C, TH], f32, tag="ps",
                                name=f"psf3{blk}{i}{th}") for i in range(4)]
                for k in range(K4):
                    ws = mk([C, 4 * C], b16, f"ws{(blk * K4 + k) % 12}")
                    eng = nc.gpsimd if k % 2 == 0 else nc.scalar
                    eng.dma_start(
                        out=ws, in_=w3T.ap()[k * C:(k + 1) * C,
                                             blk * 4 * C:(blk + 1) * 4 * C])
                    for i in range(4):
                        nc.tensor.matmul(pf[i], ws[:, i * C:(i + 1) * C],
                                         f2[k], start=(k == 0),
                                         stop=(k == K4 - 1))
                for i in range(4):
                    c = blk * 4 + i
                    t = mk([C, TH], f32, f"r2o{c}")
                    nc.vector.scalar_tensor_tensor(t, pf[i], col("b3", c),
                                                   hf[c], op0=Alu.add,
                                                   op1=Alu.add)
                    r2[c] = t

            # --- LN2 -> out, DMA ---
            of, _ = layer_norm(r2, "l2w", "l2b",
                               lambda c: mk([C, TH], f32, f"r2o{c}"), False)
            for c in range(KD):
                nc.sync.dma_start(out=outT.ap()[c * C:(c + 1) * C, tsl],
                                  in_=of[c])

        psum.release()
        P.release()

    nc.compile()
    return nc


def _get_nc():
    if "nc" not in _nc_cache:
        _nc_cache["nc"] = _build()
    return _nc_cache["nc"]


def kernel(x, mask, Wq, bq, Wk, bk, Wv, bv, Wo, bo, ln1_w, ln1_b,
           W1, b1, W2, b2, W3, b3, ln2_w, ln2_b):
    from concourse.bass_utils import run_bass_kernel_spmd

    bf = ml_dtypes.bfloat16
    f32 = np.float32

    assert np.all(np.asarray(mask) != 0), \
        "kernel specialized for the all-ones mask this module is run with"

    x = np.asarray(x, f32)

    def chunks(v, n):
        v = np.asarray(v, f32).reshape(n, C)
        return [v[i] for i in range(n)]

    cols = []
    for (nm, (_st, cnt)), src in zip(
            _BCOLS.items(),
            [np.asarray(bq, f32) * SCALE, bk, bo, b1, b2, b3,
             ln1_w, ln1_b, ln2_w, ln2_b]):
        cols += chunks(src, cnt)
    bcols = np.stack(cols, axis=1).astype(f32)  # [C, NBCOL]

    shared = {
        "wqT": np.ascontiguousarray(np.asarray(Wq, f32).T.astype(bf)),
        "wkT": np.ascontiguousarray(np.asarray(Wk, f32).T.astype(bf)),
        "wvT": np.ascontiguousarray(np.asarray(Wv, f32).T.astype(bf)),
        "woT": np.ascontiguousarray(np.asarray(Wo, f32).T.astype(bf)),
        "w1T": np.ascontiguousarray(np.asarray(W1, f32).T.astype(bf)),
        "w2T": np.ascontiguousarray(np.asarray(W2, f32).T.astype(bf)),
        "w3T": np.ascontiguousarray(np.asarray(W3, f32).T.astype(bf)),
        "bcols": bcols,
        "bv_": np.asarray(bv, f32).reshape(1, D),
    }

    in_maps = []
    for c in range(NCORES):
        xc = np.ascontiguousarray(
            x[c * BPC:(c + 1) * BPC].reshape(TOK, D).T)
        m = dict(shared)
        m["xT"] = xc
        m["xTb"] = xc.astype(bf)
        in_maps.append(m)

    nc = _get_nc()
    res = run_bass_kernel_spmd(nc, in_maps, core_ids=list(range(NCORES)),
                               trace=_trace["on"])
    _trace["res"] = res

    out = np.empty((B, S, D), f32)
    for c in range(NCORES):
        out[c * BPC:(c + 1) * BPC] = np.asarray(
            res.results[c]["outT"]).T.reshape(BPC, S, D)
    return out


# revision 23
# speedup vs baseline: 1.0120x; 1.0120x over previous
"""Fused decoder-layer kernel for Trainium2 (8 NeuronCores, data-parallel over batch).

Self-contained: hardcodes shapes B=64, S=128, D=1024, H=8, DK=128, DFF=2048.

Strategy:
  - Shard batch 8-ways (8 batches = 1024 tokens per core). No collectives.
  - Activations kept feature-major ("transposed", [feat, tok]) on device so
    every matmul Y = X @ W.T becomes Y.T with the contraction dim on SBUF
    partitions; weights are pre-transposed on host and cast bf16.
  - The layer is processed in two token-halves of 512 (4 batches each);
    attention and layernorm are token-local, so the halves are independent
    pipelines that share SBUF slots (Tile inserts the waits).
  - Attention without transposes: scoresT[k,q] = K_h^T-free matmul, exp on
    ScalarE (shift-free softmax: logits are O(1) for this module), both
    V-matmuls use token-major V as the stationary operand, and the softmax
    normalization (per-q column scale) is applied at the final drain using a
    GpSimd partition-all-reduce of exp(scores). One PSUM bank per unit
    (scores/av/ctx packed as column slices) keeps many units in flight.
  - LayerNorm over features (= partitions) via PE ones-reduction for
    sum / sum-of-squares, GpSimd partition_broadcast for mean/invstd rows.
  - Residuals and layernorm in fp32; matmul operands bf16 (fp32 PSUM accum).
  - Streamed FFN weights are DMA'd from the GpSimd DGE (the Sync DGE is a
    serial ~0.6us-per-descriptor bottleneck); everything else from Sync.
"""

import sys

if "/opt/trn_rl_repo" not in sys.path:
    sys.path.insert(0, "/opt/trn_rl_repo")

import numpy as np
import ml_dtypes

B, S, D, H = 64, 128, 1024, 8
DK = D // H
DFF = 2048
D4 = 4 * D
EPS = 1e-12
NCORES = 8
BPC = B // NCORES          # batches per core
TOK = BPC * S              # tokens per core = 1024
TH = 512                   # token half processed per pipeline pass
NTH = TOK // TH            # 2
BPT = TH // S              # batches per half = 4
C = 128                    # chunk / partition size
KD = D // C                # 8
KFF = DFF // C             # 16
K4 = D4 // C               # 32
SCALE = 1.0 / np.sqrt(DK)

# packed bias/ln columns: name -> (start, count)
_BCOLS = {}
_off = 0
for _nm, _cnt in [("bqs", KD), ("bk", KD), ("bo", KD), ("b1", KFF),
                  ("b2", K4), ("b3", KD), ("l1w", KD), ("l1b", KD),
                  ("l2w", KD), ("l2b", KD)]:
    _BCOLS[_nm] = (_off, _cnt)
    _off += _cnt
NBCOL = _off

_nc_cache = {}
_trace = {"on": False, "res": None}


def _build():
    import concourse.bass as bass
    import concourse.mybir as mybir
    import concourse.tile as tile
    import concourse.bass_isa as bass_isa
    from concourse import bacc

    f32 = mybir.dt.float32
    b16 = mybir.dt.bfloat16
    Alu = mybir.AluOpType
    Act = mybir.ActivationFunctionType
    AX = mybir.AxisListType

    nc = bacc.Bacc("TRN2", target_bir_lowering=False, debug=False,
                   num_devices=NCORES)

    # ---- DRAM I/O ------------------------------------------------------
    xT = nc.dram_tensor("xT", [D, TOK], f32, kind="ExternalInput")
    xTb = nc.dram_tensor("xTb", [D, TOK], b16, kind="ExternalInput")
    wqT = nc.dram_tensor("wqT", [D, D], b16, kind="ExternalInput")
    wkT = nc.dram_tensor("wkT", [D, D], b16, kind="ExternalInput")
    wvT = nc.dram_tensor("wvT", [D, D], b16, kind="ExternalInput")
    woT = nc.dram_tensor("woT", [D, D], b16, kind="ExternalInput")
    w1T = nc.dram_tensor("w1T", [D, DFF], b16, kind="ExternalInput")
    w2T = nc.dram_tensor("w2T", [DFF, D4], b16, kind="ExternalInput")
    w3T = nc.dram_tensor("w3T", [D4, D], b16, kind="ExternalInput")
    bcols = nc.dram_tensor("bcols", [C, NBCOL], f32, kind="ExternalInput")
    bv_ = nc.dram_tensor("bv_", [1, D], f32, kind="ExternalInput")
    outT = nc.dram_tensor("outT", [D, TOK], f32, kind="ExternalOutput")

    with tile.TileContext(nc) as tc:
        P = tc.alloc_tile_pool(name="main", bufs=1)
        psum = tc.alloc_tile_pool(name="psum", bufs=8, space="PSUM")

        def mk(shape, dtype, tag):
            return P.tile(shape, dtype, tag=tag, name=tag)

        ones = mk([C, 1], b16, "ones")
        nc.vector.memset(ones, 1.0)
        eps1 = mk([1, 1], f32, "eps1")
        nc.vector.memset(eps1, EPS)

        sb_bc = mk([C, NBCOL], f32, "bcols")
        nc.sync.dma_start(out=sb_bc, in_=bcols.ap())

        def col(name, c):
            s, n = _BCOLS[name]
            assert c < n
            return sb_bc[:, s + c:s + c + 1]

        sb_bvb = []
        for oh in range(NTH):
            t = mk([C, TH], f32, f"bvb{oh}")
            src = bv_.ap()[0:1, oh * TH:(oh + 1) * TH]
            nc.sync.dma_start(out=t, in_=src.partition_broadcast(C))
            sb_bvb.append(t)

        # ================= per-token-half pipeline =================
        for th in range(NTH):
            tsl = slice(th * TH, (th + 1) * TH)

            # --- load x (bf16 for matmuls, fp32 for residual) ---
            # --- V = x @ Wv.T + bv (token-major [tok, feat]) ---
            xb, wv_t = [], []
            for k in range(KD):
                t = mk([C, TH], b16, f"xb{k}")
                nc.sync.dma_start(out=t, in_=xTb.ap()[k * C:(k + 1) * C, tsl])
                xb.append(t)
                t = mk([C, D], b16, f"swA{k}")
                nc.sync.dma_start(out=t, in_=wvT.ap()[k * C:(k + 1) * C, :])
                wv_t.append(t)
            sv = [[None] * NTH for _ in range(BPT)]
            for tb in range(BPT):
                for oh in range(NTH):
                    pv = psum.tile([C, TH], f32, tag="ps",
                                   name=f"psv{tb}{oh}{th}")
                    for k in range(KD):
                        nc.tensor.matmul(
                            pv, xb[k][:, tb * C:(tb + 1) * C],
                            wv_t[k][:, oh * TH:(oh + 1) * TH],
                            start=(k == 0), stop=(k == KD - 1))
                    t = mk([C, TH], b16, f"sv{tb}_{oh}")
                    nc.vector.tensor_add(t, pv, sb_bvb[oh])
                    sv[tb][oh] = t

            # --- Q, K for all heads (feature-major [dk, tok]) ---
            wq_t, wk_t = [], []
            for k in range(KD):
                t = mk([C, D], b16, f"swA{8 + k}")
                nc.sync.dma_start(out=t, in_=wqT.ap()[k * C:(k + 1) * C, :])
                wq_t.append(t)
            for k in range(KD):
                t = mk([C, D], b16, f"swA{k}")  # reuse wv slots
                nc.sync.dma_start(out=t, in_=wkT.ap()[k * C:(k + 1) * C, :])
                wk_t.append(t)
            qh, kh = [None] * H, [None] * H
            for h in range(H):
                pq = psum.tile([C, TH], f32, tag="ps", name=f"psq{h}{th}")
                for k in range(KD):
                    nc.tensor.matmul(pq, wq_t[k][:, h * C:(h + 1) * C],
                                     xb[k], start=(k == 0), stop=(k == KD - 1))
                qh[h] = mk([C, TH], b16, f"q{h}")
                nc.scalar.activation(qh[h], pq, Act.Identity,
                                     bias=col("bqs", h), scale=SCALE)
                pk = psum.tile([C, TH], f32, tag="ps", name=f"psk{h}{th}")
                for k in range(KD):
                    nc.tensor.matmul(pk, wk_t[k][:, h * C:(h + 1) * C],
                                     xb[k], start=(k == 0), stop=(k == KD - 1))
                kh[h] = mk([C, TH], b16, f"k{h}")
                nc.scalar.activation(kh[h], pk, Act.Identity,
                                     bias=col("bk", h))

            # --- attention units (shift-free softmax, no transposes) ---
            ctx = [None] * KD
            for h in range(KD):
                ctx[h] = mk([C, TH], b16, f"ctx{h}")
            uidx = 0
            for h in range(H):
                rrow = mk([1, TH], f32, f"rro{h % 4}")
                cxu = mk([C, TH], b16, f"ctxu{h % 4}")
                prow = psum.tile([1, TH], f32, tag="ps", name=f"prow{h}{th}")
                for tb in range(BPT):
                    u = uidx % 6
                    uidx += 1
                    bsl = slice(tb * C, (tb + 1) * C)
                    vsl = sv[tb][h // BPT][:, (h % BPT) * C:(h % BPT + 1) * C]
                    pb = psum.tile([C, 3 * C], f32, tag="ps",
                                   name=f"pat{h}{tb}{th}")
                    # scoresT[k,q] (pre-scaled via Q)
                    nc.tensor.matmul(pb[:, 0:C], kh[h][:, bsl], qh[h][:, bsl],
                                     start=True, stop=True)
                    e = mk([C, C], b16, f"e{u}")
                    nc.scalar.activation(e, pb[:, 0:C], Act.Exp)
                    # column sums via PE ones-reduce into the per-head row
                    nc.tensor.matmul(prow[0:1, bsl], ones, e,
                                     start=True, stop=True)
                    # avT_unnorm = V^T-free matmul with E
                    nc.tensor.matmul(pb[:, C:2 * C], vsl, e,
                                     start=True, stop=True)
                    avb = mk([C, C], b16, f"avb{u}")
                    nc.scalar.copy(avb, pb[:, C:2 * C])
                    # ctxT_unnorm; normalization deferred to per-head pass
                    nc.tensor.matmul(pb[:, 2 * C:3 * C], vsl, avb,
                                     start=True, stop=True)
                    nc.scalar.copy(cxu[:, bsl], pb[:, 2 * C:3 * C])
                # normalize the head's ctx columns by 1/rowsum
                nc.vector.reciprocal(rrow, prow)
                rall = mk([C, TH], f32, f"rall{h % 4}")
                nc.gpsimd.partition_broadcast(rall, rrow)
                nc.vector.tensor_mul(ctx[h], cxu, rall)

            # --- attn_out = ctx @ Wo.T + bo ; r1 = x + attn_out ---
            wo_t = []
            for k in range(KD):
                t = mk([C, D], b16, f"swA{8 + k}")  # reuse wq slots
                nc.sync.dma_start(out=t, in_=woT.ap()[k * C:(k + 1) * C, :])
                wo_t.append(t)
            xf = []
            for k in range(KD):
                t = mk([C, TH], f32, f"xf{k}")
                nc.sync.dma_start(out=t, in_=xT.ap()[k * C:(k + 1) * C, tsl])
                xf.append(t)
            r1 = [None] * KD
            for c in range(KD):
                po = psum.tile([C, TH], f32, tag="ps", name=f"pso{c}{th}")
                for k in range(KD):
                    nc.tensor.matmul(po, wo_t[k][:, c * C:(c + 1) * C],
                                     ctx[k], start=(k == 0), stop=(k == KD - 1))
                t = mk([C, TH], f32, f"r1h{c}")
                nc.vector.scalar_tensor_tensor(t, po, col("bo", c), xf[c],
                                               op0=Alu.add, op1=Alu.add)
                r1[c] = t

            # --- layernorm helper (over features = partitions) ---
            def layer_norm(src, wname, bname, out_mk, also_b16):
                vb, sq = [], []
                for c in range(KD):
                    tvb = mk([C, TH], b16, f"svb{c}")
                    nc.vector.tensor_copy(tvb, src[c])
                    vb.append(tvb)
                    tsq = mk([C, TH], b16, f"ssq{c}")
                    nc.scalar.activation(tsq, src[c], Act.Square)
                    sq.append(tsq)
                ps1 = psum.tile([1, TH], f32, tag="ps", name=f"ps1{th}")
                for c in range(KD):
                    nc.tensor.matmul(ps1, ones, vb[c], start=(c == 0),
                                     stop=(c == KD - 1))
                ps2 = psum.tile([1, TH], f32, tag="ps", name=f"ps2{th}")
                for c in range(KD):
                    nc.tensor.matmul(ps2, ones, sq[c], start=(c == 0),
                                     stop=(c == KD - 1))
                mu = mk([1, TH], f32, "rowA")
                nc.scalar.mul(mu, ps1, 1.0 / D)
                msq = mk([1, TH], f32, "rowB")
                nc.scalar.mul(msq, ps2, 1.0 / D)
                mu2 = mk([1, TH], f32, "rowC")
                nc.vector.tensor_mul(mu2, mu, mu)
                var = mk([1, TH], f32, "rowD")
                nc.vector.tensor_sub(var, msq, mu2)
                sd = mk([1, TH], f32, "rowC")
                nc.scalar.activation(sd, var, Act.Sqrt, bias=eps1)
                rinv = mk([1, TH], f32, "rowB")
                nc.vector.reciprocal(rinv, sd)
                mub = mk([C, TH], f32, "mub")
                nc.gpsimd.partition_broadcast(mub, mu)
                rb = mk([C, TH], f32, "rb")
                nc.gpsimd.partition_broadcast(rb, rinv)
                outs_f, outs_b = [], []
                for c in range(KD):
                    t1 = mk([C, TH], f32, "t1")
                    nc.vector.tensor_sub(t1, src[c], mub)
                    t2 = mk([C, TH], f32, f"t2{c % 2}")
                    nc.vector.tensor_mul(t2, t1, rb)
                    to = out_mk(c)
                    nc.scalar.activation(to, t2, Act.Identity,
                                         bias=col(bname, c),
                                         scale=col(wname, c))
                    outs_f.append(to)
                    if also_b16:
                        tb16 = mk([C, TH], b16, f"sv{c // 2}_{c % 2}")
                        nc.vector.tensor_copy(tb16, to)
                        outs_b.append(tb16)
                return outs_f, outs_b

            # --- LN1: h fp32 (r1 slots) + bf16 copy (sv slots) ---
            hf, hb = layer_norm(r1, "l1w", "l1b",
                                lambda c: mk([C, TH], f32, f"r1h{c}"), True)

            # --- f1 = relu(h @ W1.T + b1); W1 streamed via GpSimd DGE ---
            f1tag = [f"ctx{i}" for i in range(8)] + [f"q{i}" for i in range(8)]
            f1 = [None] * KFF
            for blk in range(KFF // 4):
                pf = [psum.tile([C, TH], f32, tag="ps",
                                name=f"psf1{blk}{i}{th}") for i in range(4)]
                for k in range(KD):
                    ws = mk([C, 4 * C], b16, f"ws{(blk * KD + k) % 12}")
                    eng = nc.gpsimd if k % 2 == 0 else nc.scalar
                    eng.dma_start(
                        out=ws, in_=w1T.ap()[k * C:(k + 1) * C,
                                             blk * 4 * C:(blk + 1) * 4 * C])
                    for i in range(4):
                        nc.tensor.matmul(pf[i], ws[:, i * C:(i + 1) * C],
                                         hb[k], start=(k == 0),
                                         stop=(k == KD - 1))
                for i in range(4):
                    c = blk * 4 + i
                    t = mk([C, TH], b16, f1tag[c])
                    nc.scalar.activation(t, pf[i], Act.Relu, bias=col("b1", c))
                    f1[c] = t

            # --- f2 = relu(f1 @ W2.T + b2) ---
            f2tag = [f"xf{i}" for i in range(8)] + \
                    [f"xb{i}" for i in range(8)] + \
                    [f"sv{i // 2}_{i % 2}" for i in range(8)] + \
                    [f"k{i}" for i in range(8)]
            f2 = [None] * K4
            for blk in range(K4 // 4):
                pf = [psum.tile([C, TH], f32, tag="ps",
                                name=f"psf2{blk}{i}{th}") for i in range(4)]
                for k in range(KFF):
                    ws = mk([C, 4 * C], b16, f"ws{(blk * KFF + k) % 12}")
                    eng = nc.gpsimd if k % 2 == 0 else nc.scalar
                    eng.dma_start(
                        out=ws, in_=w2T.ap()[k * C:(k + 1) * C,
                                             blk * 4 * C:(blk + 1) * 4 * C])
                    for i in range(4):
                        nc.tensor.matmul(pf[i], ws[:, i * C:(i + 1) * C],
                                         f1[k], start=(k == 0),
                                         stop=(k == KFF - 1))
                for i in range(4):
                    c = blk * 4 + i
                    t = mk([C, TH], b16, f2tag[c])
                    nc.scalar.activation(t, pf[i], Act.Relu, bias=col("b2", c))
                    f2[c] = t

            # --- f3 = f2 @ W3.T + b3 ; r2 = h + f3 ---
            r2 = [None] * KD
            for blk in range(KD // 4):
                pf = [psum.tile([C, TH], f32, tag="ps",
                                name=f"psf3{blk}{i}{th}") for i in range(4)]
                for k in range(K4):
                    ws = mk([C, 4 * C], b16, f"ws{(blk * K4 + k) % 12}")
                    eng = nc.gpsimd if k % 2 == 0 else nc.scalar
                    eng.dma_start(
                        out=ws, in_=w3T.ap()[k * C:(k + 1) * C,
                                             blk * 4 * C:(blk + 1) * 4 * C])
                    for i in range(4):
                        nc.tensor.matmul(pf[i], ws[:, i * C:(i + 1) * C],
                                         f2[k], start=(k == 0),
                                         stop=(k == K4 - 1))
                for i in range(4):
                    c = blk * 4 + i
                    t = mk([C, TH], f32, f"r2o{c}")
                    nc.vector.scalar_tensor_tensor(t, pf[i], col("b3", c),
                                                   hf[c], op0=Alu.add,
                                                   op1=Alu.add)
                    r2[c] = t

            # --- LN2 -> out, DMA ---
            of, _ = layer_norm(r2, "l2w", "l2b",
                               lambda c: mk([C, TH], f32, f"r2o{c}"), False)
            for c in range(KD):
                nc.sync.dma_start(out=outT.ap()[c * C:(c + 1) * C, tsl],
                                  in_=of[c])

        psum.release()
        P.release()

    nc.compile()
    return nc


def _get_nc():
    if "nc" not in _nc_cache:
        _nc_cache["nc"] = _build()
    return _nc_cache["nc"]


def kernel(x, mask, Wq, bq, Wk, bk, Wv, bv, Wo, bo, ln1_w, ln1_b,
           W1, b1, W2, b2, W3, b3, ln2_w, ln2_b):
    from concourse.bass_utils import run_bass_kernel_spmd

    bf = ml_dtypes.bfloat16
    f32 = np.float32

    assert np.all(np.asarray(mask) != 0), \
        "kernel specialized for the all-ones mask this module is run with"

    x = np.asarray(x, f32)

    def chunks(v, n):
        v = np.asarray(v, f32).reshape(n, C)
        return [v[i] for i in range(n)]

    cols = []
    for (nm, (_st, cnt)), src in zip(
            _BCOLS.items(),
            [np.asarray(bq, f32) * SCALE, bk, bo, b1, b2, b3,
             ln1_w, ln1_b, ln2_w, ln2_b]):
        cols += chunks(src, cnt)
    bcols = np.stack(cols, axis=1).astype(f32)  # [C, NBCOL]

    shared = {
        "wqT": np.ascontiguousarray(np.asarray(Wq, f32).T.astype(bf)),
        "wkT": np.ascontiguousarray(np.asarray(Wk, f32).T.astype(bf)),
        "wvT": np.ascontiguousarray(np.asarray(Wv, f32).T.astype(bf)),
        "woT": np.ascontiguousarray(np.asarray(Wo, f32).T.astype(bf)),
        "w1T": np.ascontiguousarray(np.asarray(W1, f32).T.astype(bf)),
        "w2T": np.ascontiguousarray(np.asarray(W2, f32).T.astype(bf)),
        "w3T": np.ascontiguousarray(np.asarray(W3, f32).T.astype(bf)),
        "bcols": bcols,
        "bv_": np.asarray(bv, f32).reshape(1, D),
    }

    in_maps = []
    for c in range(NCORES):
        xc = np.ascontiguousarray(
            x[c * BPC:(c + 1) * BPC].reshape(TOK, D).T)
        m = dict(shared)
        m["xT"] = xc
        m["xTb"] = xc.astype(bf)
        in_maps.append(m)

    nc = _get_nc()
    res = run_bass_kernel_spmd(nc, in_maps, core_ids=list(range(NCORES)),
                               trace=_trace["on"])
    _trace["res"] = res

    out = np.empty((B, S, D), f32)
    for c in range(NCORES):
        out[c * BPC:(c + 1) * BPC] = np.asarray(
            res.results[c]["outT"]).T.reshape(BPC, S, D)
    return out


# revision 24
# speedup vs baseline: 1.0271x; 1.0149x over previous
"""Fused decoder-layer kernel for Trainium2 (8 NeuronCores, data-parallel over batch).

Self-contained: hardcodes shapes B=64, S=128, D=1024, H=8, DK=128, DFF=2048.

Strategy:
  - Shard batch 8-ways (8 batches = 1024 tokens per core). No collectives.
  - Activations kept feature-major ("transposed", [feat, tok]) on device so
    every matmul Y = X @ W.T becomes Y.T with the contraction dim on SBUF
    partitions; weights are pre-transposed on host and cast bf16.
  - The layer is processed in two token-halves of 512 (4 batches each);
    attention and layernorm are token-local, so the halves are independent
    pipelines that share SBUF slots (Tile inserts the waits).
  - Attention without transposes: scoresT[k,q] = K_h^T-free matmul, exp on
    ScalarE (shift-free softmax: logits are O(1) for this module), both
    V-matmuls use token-major V as the stationary operand, and the softmax
    normalization (per-q column scale) is applied at the final drain using a
    GpSimd partition-all-reduce of exp(scores). One PSUM bank per unit
    (scores/av/ctx packed as column slices) keeps many units in flight.
  - LayerNorm over features (= partitions) via PE ones-reduction for
    sum / sum-of-squares, GpSimd partition_broadcast for mean/invstd rows.
  - Residuals and layernorm in fp32; matmul operands bf16 (fp32 PSUM accum).
  - Streamed FFN weights are DMA'd from the GpSimd DGE (the Sync DGE is a
    serial ~0.6us-per-descriptor bottleneck); everything else from Sync.
"""

import sys

if "/opt/trn_rl_repo" not in sys.path:
    sys.path.insert(0, "/opt/trn_rl_repo")

import numpy as np
import ml_dtypes

B, S, D, H = 64, 128, 1024, 8
DK = D // H
DFF = 2048
D4 = 4 * D
EPS = 1e-12
NCORES = 8
BPC = B // NCORES          # batches per core
TOK = BPC * S              # tokens per core = 1024
TH = 512                   # token half processed per pipeline pass
NTH = TOK // TH            # 2
BPT = TH // S              # batches per half = 4
C = 128                    # chunk / partition size
KD = D // C                # 8
KFF = DFF // C             # 16
K4 = D4 // C               # 32
SCALE = 1.0 / np.sqrt(DK)

# packed bias/ln columns: name -> (start, count)
_BCOLS = {}
_off = 0
for _nm, _cnt in [("bqs", KD), ("bk", KD), ("bo", KD), ("b1", KFF),
                  ("b2", K4), ("b3", KD), ("l1w", KD), ("l1b", KD),
                  ("l2w", KD), ("l2b", KD)]:
    _BCOLS[_nm] = (_off, _cnt)
    _off += _cnt
NBCOL = _off

_nc_cache = {}
_trace = {"on": False, "res": None}


def _build():
    import concourse.bass as bass
    import concourse.mybir as mybir
    import concourse.tile as tile
    import concourse.bass_isa as bass_isa
    from concourse import bacc

    f32 = mybir.dt.float32
    b16 = mybir.dt.bfloat16
    Alu = mybir.AluOpType
    Act = mybir.ActivationFunctionType
    AX = mybir.AxisListType

    nc = bacc.Bacc("TRN2", target_bir_lowering=False, debug=False,
                   num_devices=NCORES)

    # ---- DRAM I/O ------------------------------------------------------
    xT = nc.dram_tensor("xT", [D, TOK], f32, kind="ExternalInput")
    xTb = nc.dram_tensor("xTb", [D, TOK], b16, kind="ExternalInput")
    wqT = nc.dram_tensor("wqT", [D, D], b16, kind="ExternalInput")
    wkT = nc.dram_tensor("wkT", [D, D], b16, kind="ExternalInput")
    wvT = nc.dram_tensor("wvT", [D, D], b16, kind="ExternalInput")
    woT = nc.dram_tensor("woT", [D, D], b16, kind="ExternalInput")
    w1T = nc.dram_tensor("w1T", [D, DFF], b16, kind="ExternalInput")
    w2T = nc.dram_tensor("w2T", [DFF, D4], b16, kind="ExternalInput")
    w3T = nc.dram_tensor("w3T", [D4, D], b16, kind="ExternalInput")
    bcols = nc.dram_tensor("bcols", [C, NBCOL], f32, kind="ExternalInput")
    bv_ = nc.dram_tensor("bv_", [1, D], f32, kind="ExternalInput")
    outT = nc.dram_tensor("outT", [D, TOK], f32, kind="ExternalOutput")

    with tile.TileContext(nc) as tc:
        P = tc.alloc_tile_pool(name="main", bufs=1)
        psum = tc.alloc_tile_pool(name="psum", bufs=8, space="PSUM")

        def mk(shape, dtype, tag):
            return P.tile(shape, dtype, tag=tag, name=tag)

        ones = mk([C, 1], b16, "ones")
        nc.vector.memset(ones, 1.0)
        eps1 = mk([1, 1], f32, "eps1")
        nc.vector.memset(eps1, EPS)

        sb_bc = mk([C, NBCOL], f32, "bcols")
        nc.sync.dma_start(out=sb_bc, in_=bcols.ap())

        def col(name, c):
            s, n = _BCOLS[name]
            assert c < n
            return sb_bc[:, s + c:s + c + 1]

        sb_bvb = []
        for oh in range(NTH):
            t = mk([C, TH], f32, f"bvb{oh}")
            src = bv_.ap()[0:1, oh * TH:(oh + 1) * TH]
            nc.sync.dma_start(out=t, in_=src.partition_broadcast(C))
            sb_bvb.append(t)

        # ================= per-token-half pipeline =================
        for th in range(NTH):
            tsl = slice(th * TH, (th + 1) * TH)

            # --- load x (bf16 for matmuls, fp32 for residual) ---
            # --- V = x @ Wv.T + bv (token-major [tok, feat]) ---
            xb, wv_t = [], []
            for k in range(KD):
                t = mk([C, TH], b16, f"xb{k}")
                nc.sync.dma_start(out=t, in_=xTb.ap()[k * C:(k + 1) * C, tsl])
                xb.append(t)
                t = mk([C, D], b16, f"swA{k}")
                nc.sync.dma_start(out=t, in_=wvT.ap()[k * C:(k + 1) * C, :])
                wv_t.append(t)
            sv = [[None] * NTH for _ in range(BPT)]
            for tb in range(BPT):
                for oh in range(NTH):
                    pv = psum.tile([C, TH], f32, tag="ps",
                                   name=f"psv{tb}{oh}{th}")
                    for k in range(KD):
                        nc.tensor.matmul(
                            pv, xb[k][:, tb * C:(tb + 1) * C],
                            wv_t[k][:, oh * TH:(oh + 1) * TH],
                            start=(k == 0), stop=(k == KD - 1))
                    t = mk([C, TH], b16, f"sv{tb}_{oh}")
                    nc.vector.tensor_add(t, pv, sb_bvb[oh])
                    sv[tb][oh] = t

            # --- Q, K for all heads (feature-major [dk, tok]) ---
            wq_t, wk_t = [], []
            for k in range(KD):
                t = mk([C, D], b16, f"swA{8 + k}")
                nc.sync.dma_start(out=t, in_=wqT.ap()[k * C:(k + 1) * C, :])
                wq_t.append(t)
            for k in range(KD):
                t = mk([C, D], b16, f"swA{k}")  # reuse wv slots
                nc.sync.dma_start(out=t, in_=wkT.ap()[k * C:(k + 1) * C, :])
                wk_t.append(t)
            qh, kh = [None] * H, [None] * H
            for h in range(H):
                pq = psum.tile([C, TH], f32, tag="ps", name=f"psq{h}{th}")
                for k in range(KD):
                    nc.tensor.matmul(pq, wq_t[k][:, h * C:(h + 1) * C],
                                     xb[k], start=(k == 0), stop=(k == KD - 1))
                qh[h] = mk([C, TH], b16, f"q{h}")
                nc.scalar.activation(qh[h], pq, Act.Identity,
                                     bias=col("bqs", h), scale=SCALE)
                pk = psum.tile([C, TH], f32, tag="ps", name=f"psk{h}{th}")
                for k in range(KD):
                    nc.tensor.matmul(pk, wk_t[k][:, h * C:(h + 1) * C],
                                     xb[k], start=(k == 0), stop=(k == KD - 1))
                kh[h] = mk([C, TH], b16, f"k{h}")
                nc.scalar.activation(kh[h], pk, Act.Identity,
                                     bias=col("bk", h))

            # --- attention units (shift-free softmax, no transposes) ---
            ctx = [None] * KD
            for h in range(KD):
                ctx[h] = mk([C, TH], b16, f"ctx{h}")
            for h in range(H):
                hsl = [slice(tb * C, (tb + 1) * C) for tb in range(BPT)]
                vsls = [sv[tb][h // BPT][:, (h % BPT) * C:(h % BPT + 1) * C]
                        for tb in range(BPT)]
                # scoresT[k,q] for all 4 batches packed into one bank
                pbs = psum.tile([C, TH], f32, tag="ps", name=f"pbs{h}{th}")
                for tb in range(BPT):
                    nc.tensor.matmul(pbs[:, hsl[tb]], kh[h][:, hsl[tb]],
                                     qh[h][:, hsl[tb]], start=True, stop=True)
                e = mk([C, TH], b16, f"eA{h % 4}")
                nc.scalar.activation(e, pbs, Act.Exp)
                # per-column sums of exp via one wide PE ones-reduce
                prow = psum.tile([1, TH], f32, tag="ps", name=f"prow{h}{th}")
                nc.tensor.matmul(prow, ones, e, start=True, stop=True)
                rrow = mk([1, TH], f32, f"rro{h % 4}")
                nc.vector.reciprocal(rrow, prow)
                # avT_unnorm per batch, packed into one bank
                pbav = psum.tile([C, TH], f32, tag="ps", name=f"pbav{h}{th}")
                for tb in range(BPT):
                    nc.tensor.matmul(pbav[:, hsl[tb]], vsls[tb],
                                     e[:, hsl[tb]], start=True, stop=True)
                avb = mk([C, TH], b16, f"avA{h % 4}")
                nc.scalar.copy(avb, pbav)
                # ctxT_unnorm per batch
                pbcx = psum.tile([C, TH], f32, tag="ps", name=f"pbcx{h}{th}")
                for tb in range(BPT):
                    nc.tensor.matmul(pbcx[:, hsl[tb]], vsls[tb],
                                     avb[:, hsl[tb]], start=True, stop=True)
                cxu = mk([C, TH], b16, f"ctxu{h % 4}")
                nc.scalar.copy(cxu, pbcx)
                # normalize the head's ctx columns by 1/rowsum
                rall = mk([C, TH], f32, f"rall{h % 4}")
                nc.gpsimd.partition_broadcast(rall, rrow)
                nc.vector.tensor_mul(ctx[h], cxu, rall)

            # --- attn_out = ctx @ Wo.T + bo ; r1 = x + attn_out ---
            wo_t = []
            for k in range(KD):
                t = mk([C, D], b16, f"swA{8 + k}")  # reuse wq slots
                nc.sync.dma_start(out=t, in_=woT.ap()[k * C:(k + 1) * C, :])
                wo_t.append(t)
            xf = []
            for k in range(KD):
                t = mk([C, TH], f32, f"xf{k}")
                nc.sync.dma_start(out=t, in_=xT.ap()[k * C:(k + 1) * C, tsl])
                xf.append(t)
            r1 = [None] * KD
            for c in range(KD):
                po = psum.tile([C, TH], f32, tag="ps", name=f"pso{c}{th}")
                for k in range(KD):
                    nc.tensor.matmul(po, wo_t[k][:, c * C:(c + 1) * C],
                                     ctx[k], start=(k == 0), stop=(k == KD - 1))
                t = mk([C, TH], f32, f"r1h{c}")
                nc.vector.scalar_tensor_tensor(t, po, col("bo", c), xf[c],
                                               op0=Alu.add, op1=Alu.add)
                r1[c] = t

            # --- layernorm helper (over features = partitions) ---
            def layer_norm(src, wname, bname, out_mk, also_b16):
                vb, sq = [], []
                for c in range(KD):
                    tvb = mk([C, TH], b16, f"svb{c}")
                    nc.vector.tensor_copy(tvb, src[c])
                    vb.append(tvb)
                    tsq = mk([C, TH], b16, f"ssq{c}")
                    nc.scalar.activation(tsq, src[c], Act.Square)
                    sq.append(tsq)
                ps1 = psum.tile([1, TH], f32, tag="ps", name=f"ps1{th}")
                for c in range(KD):
                    nc.tensor.matmul(ps1, ones, vb[c], start=(c == 0),
                                     stop=(c == KD - 1))
                ps2 = psum.tile([1, TH], f32, tag="ps", name=f"ps2{th}")
                for c in range(KD):
                    nc.tensor.matmul(ps2, ones, sq[c], start=(c == 0),
                                     stop=(c == KD - 1))
                mu = mk([1, TH], f32, "rowA")
                nc.scalar.mul(mu, ps1, 1.0 / D)
                msq = mk([1, TH], f32, "rowB")
                nc.scalar.mul(msq, ps2, 1.0 / D)
                mu2 = mk([1, TH], f32, "rowC")
                nc.vector.tensor_mul(mu2, mu, mu)
                var = mk([1, TH], f32, "rowD")
                nc.vector.tensor_sub(var, msq, mu2)
                sd = mk([1, TH], f32, "rowC")
                nc.scalar.activation(sd, var, Act.Sqrt, bias=eps1)
                rinv = mk([1, TH], f32, "rowB")
                nc.vector.reciprocal(rinv, sd)
                mub = mk([C, TH], f32, "mub")
                nc.gpsimd.partition_broadcast(mub, mu)
                rb = mk([C, TH], f32, "rb")
                nc.gpsimd.partition_broadcast(rb, rinv)
                outs_f, outs_b = [], []
                for c in range(KD):
                    t1 = mk([C, TH], f32, "t1")
                    nc.vector.tensor_sub(t1, src[c], mub)
                    t2 = mk([C, TH], f32, f"t2{c % 2}")
                    nc.vector.tensor_mul(t2, t1, rb)
                    to = out_mk(c)
                    nc.scalar.activation(to, t2, Act.Identity,
                                         bias=col(bname, c),
                                         scale=col(wname, c))
                    outs_f.append(to)
                    if also_b16:
                        tb16 = mk([C, TH], b16, f"sv{c // 2}_{c % 2}")
                        nc.vector.tensor_copy(tb16, to)
                        outs_b.append(tb16)
                return outs_f, outs_b

            # --- LN1: h fp32 (r1 slots) + bf16 copy (sv slots) ---
            hf, hb = layer_norm(r1, "l1w", "l1b",
                                lambda c: mk([C, TH], f32, f"r1h{c}"), True)

            # --- f1 = relu(h @ W1.T + b1); W1 streamed via GpSimd DGE ---
            f1tag = [f"ctx{i}" for i in range(8)] + [f"q{i}" for i in range(8)]
            f1 = [None] * KFF
            for blk in range(KFF // 4):
                pf = [psum.tile([C, TH], f32, tag="ps",
                                name=f"psf1{blk}{i}{th}") for i in range(4)]
                for k in range(KD):
                    ws = mk([C, 4 * C], b16, f"ws{(blk * KD + k) % 12}")
                    eng = nc.gpsimd if k % 2 == 0 else nc.scalar
                    eng.dma_start(
                        out=ws, in_=w1T.ap()[k * C:(k + 1) * C,
                                             blk * 4 * C:(blk + 1) * 4 * C])
                    for i in range(4):
                        nc.tensor.matmul(pf[i], ws[:, i * C:(i + 1) * C],
                                         hb[k], start=(k == 0),
                                         stop=(k == KD - 1))
                for i in range(4):
                    c = blk * 4 + i
                    t = mk([C, TH], b16, f1tag[c])
                    nc.scalar.activation(t, pf[i], Act.Relu, bias=col("b1", c))
                    f1[c] = t

            # --- f2 = relu(f1 @ W2.T + b2) ---
            f2tag = [f"xf{i}" for i in range(8)] + \
                    [f"xb{i}" for i in range(8)] + \
                    [f"sv{i // 2}_{i % 2}" for i in range(8)] + \
                    [f"k{i}" for i in range(8)]
            f2 = [None] * K4
            for blk in range(K4 // 4):
                pf = [psum.tile([C, TH], f32, tag="ps",
                                name=f"psf2{blk}{i}{th}") for i in range(4)]
                for k in range(KFF):
                    ws = mk([C, 4 * C], b16, f"ws{(blk * KFF + k) % 12}")
                    eng = nc.gpsimd if k % 2 == 0 else nc.scalar
                    eng.dma_start(
                        out=ws, in_=w2T.ap()[k * C:(k + 1) * C,
                                             blk * 4 * C:(blk + 1) * 4 * C])
                    for i in range(4):
                        nc.tensor.matmul(pf[i], ws[:, i * C:(i + 1) * C],
                                         f1[k], start=(k == 0),
                                         stop=(k == KFF - 1))
                for i in range(4):
                    c = blk * 4 + i
                    t = mk([C, TH], b16, f2tag[c])
                    nc.scalar.activation(t, pf[i], Act.Relu, bias=col("b2", c))
                    f2[c] = t

            # --- f3 = f2 @ W3.T + b3 ; r2 = h + f3 ---
            r2 = [None] * KD
            for blk in range(KD // 4):
                pf = [psum.tile([C, TH], f32, tag="ps",
                                name=f"psf3{blk}{i}{th}") for i in range(4)]
                for k in range(K4):
                    ws = mk([C, 4 * C], b16, f"ws{(blk * K4 + k) % 12}")
                    eng = nc.gpsimd if k % 2 == 0 else nc.scalar
                    eng.dma_start(
                        out=ws, in_=w3T.ap()[k * C:(k + 1) * C,
                                             blk * 4 * C:(blk + 1) * 4 * C])
                    for i in range(4):
                        nc.tensor.matmul(pf[i], ws[:, i * C:(i + 1) * C],
                                         f2[k], start=(k == 0),
                                         stop=(k == K4 - 1))
                for i in range(4):
                    c = blk * 4 + i
                    t = mk([C, TH], f32, f"r2o{c}")
                    nc.vector.scalar_tensor_tensor(t, pf[i], col("b3", c),
                                                   hf[c], op0=Alu.add,
                                                   op1=Alu.add)
                    r2[c] = t

            # --- LN2 -> out, DMA ---
            of, _ = layer_norm(r2, "l2w", "l2b",
                               lambda c: mk([C, TH], f32, f"r2o{c}"), False)
            for c in range(KD):
                nc.sync.dma_start(out=outT.ap()[c * C:(c + 1) * C, tsl],
                                  in_=of[c])

        psum.release()
        P.release()

    nc.compile()
    return nc


def _get_nc():
    if "nc" not in _nc_cache:
        _nc_cache["nc"] = _build()
    return _nc_cache["nc"]


def kernel(x, mask, Wq, bq, Wk, bk, Wv, bv, Wo, bo, ln1_w, ln1_b,
           W1, b1, W2, b2, W3, b3, ln2_w, ln2_b):
    from concourse.bass_utils import run_bass_kernel_spmd

    bf = ml_dtypes.bfloat16
    f32 = np.float32

    assert np.all(np.asarray(mask) != 0), \
        "kernel specialized for the all-ones mask this module is run with"

    x = np.asarray(x, f32)

    def chunks(v, n):
        v = np.asarray(v, f32).reshape(n, C)
        return [v[i] for i in range(n)]

    cols = []
    for (nm, (_st, cnt)), src in zip(
            _BCOLS.items(),
            [np.asarray(bq, f32) * SCALE, bk, bo, b1, b2, b3,
             ln1_w, ln1_b, ln2_w, ln2_b]):
        cols += chunks(src, cnt)
    bcols = np.stack(cols, axis=1).astype(f32)  # [C, NBCOL]

    shared = {
        "wqT": np.ascontiguousarray(np.asarray(Wq, f32).T.astype(bf)),
        "wkT": np.ascontiguousarray(np.asarray(Wk, f32).T.astype(bf)),
        "wvT": np.ascontiguousarray(np.asarray(Wv, f32).T.astype(bf)),
        "woT": np.ascontiguousarray(np.asarray(Wo, f32).T.astype(bf)),
        "w1T": np.ascontiguousarray(np.asarray(W1, f32).T.astype(bf)),
        "w2T": np.ascontiguousarray(np.asarray(W2, f32).T.astype(bf)),
        "w3T": np.ascontiguousarray(np.asarray(W3, f32).T.astype(bf)),
        "bcols": bcols,
        "bv_": np.asarray(bv, f32).reshape(1, D),
    }

    in_maps = []
    for c in range(NCORES):
        xc = np.ascontiguousarray(
            x[c * BPC:(c + 1) * BPC].reshape(TOK, D).T)
        m = dict(shared)
        m["xT"] = xc
        m["xTb"] = xc.astype(bf)
        in_maps.append(m)

    nc = _get_nc()
    res = run_bass_kernel_spmd(nc, in_maps, core_ids=list(range(NCORES)),
                               trace=_trace["on"])
    _trace["res"] = res

    out = np.empty((B, S, D), f32)
    for c in range(NCORES):
        out[c * BPC:(c + 1) * BPC] = np.asarray(
            res.results[c]["outT"]).T.reshape(BPC, S, D)
    return out


# revision 26
# speedup vs baseline: 1.0440x; 1.0165x over previous
"""Fused decoder-layer kernel for Trainium2 (8 NeuronCores, data-parallel over batch).

Self-contained: hardcodes shapes B=64, S=128, D=1024, H=8, DK=128, DFF=2048.

Strategy:
  - Shard batch 8-ways (8 batches = 1024 tokens per core). No collectives.
  - Activations kept feature-major ("transposed", [feat, tok]) on device so
    every matmul Y = X @ W.T becomes Y.T with the contraction dim on SBUF
    partitions; weights are pre-transposed on host and cast bf16.
  - The layer is processed in two token-halves of 512 (4 batches each);
    attention and layernorm are token-local, so the halves are independent
    pipelines that share SBUF slots (Tile inserts the waits).
  - Attention without transposes: scoresT[k,q] = K_h^T-free matmul, exp on
    ScalarE (shift-free softmax: logits are O(1) for this module), both
    V-matmuls use token-major V as the stationary operand, and the softmax
    normalization (per-q column scale) is applied at the final drain using a
    GpSimd partition-all-reduce of exp(scores). One PSUM bank per unit
    (scores/av/ctx packed as column slices) keeps many units in flight.
  - LayerNorm over features (= partitions) via PE ones-reduction for
    sum / sum-of-squares, GpSimd partition_broadcast for mean/invstd rows.
  - Residuals and layernorm in fp32; matmul operands bf16 (fp32 PSUM accum).
  - Streamed FFN weights are DMA'd from the GpSimd DGE (the Sync DGE is a
    serial ~0.6us-per-descriptor bottleneck); everything else from Sync.
"""

import sys

if "/opt/trn_rl_repo" not in sys.path:
    sys.path.insert(0, "/opt/trn_rl_repo")

import numpy as np
import ml_dtypes

B, S, D, H = 64, 128, 1024, 8
DK = D // H
DFF = 2048
D4 = 4 * D
EPS = 1e-12
NCORES = 8
BPC = B // NCORES          # batches per core
TOK = BPC * S              # tokens per core = 1024
TH = 512                   # token half processed per pipeline pass
NTH = TOK // TH            # 2
BPT = TH // S              # batches per half = 4
C = 128                    # chunk / partition size
KD = D // C                # 8
KFF = DFF // C             # 16
K4 = D4 // C               # 32
SCALE = 1.0 / np.sqrt(DK)

# packed bias/ln columns: name -> (start, count)
_BCOLS = {}
_off = 0
for _nm, _cnt in [("bqs", KD), ("bk", KD), ("bo", KD), ("b1", KFF),
                  ("b2", K4), ("b3", KD), ("l1w", KD), ("l1b", KD),
                  ("l2w", KD), ("l2b", KD)]:
    _BCOLS[_nm] = (_off, _cnt)
    _off += _cnt
NBCOL = _off

_nc_cache = {}
_trace = {"on": False, "res": None}


def _build():
    import concourse.bass as bass
    import concourse.mybir as mybir
    import concourse.tile as tile
    import concourse.bass_isa as bass_isa
    from concourse import bacc

    f32 = mybir.dt.float32
    b16 = mybir.dt.bfloat16
    Alu = mybir.AluOpType
    Act = mybir.ActivationFunctionType
    AX = mybir.AxisListType

    nc = bacc.Bacc("TRN2", target_bir_lowering=False, debug=False,
                   num_devices=NCORES)

    # ---- DRAM I/O ------------------------------------------------------
    xT = nc.dram_tensor("xT", [D, TOK], f32, kind="ExternalInput")
    xTb = nc.dram_tensor("xTb", [D, TOK], b16, kind="ExternalInput")
    wqT = nc.dram_tensor("wqT", [D, D], b16, kind="ExternalInput")
    wkT = nc.dram_tensor("wkT", [D, D], b16, kind="ExternalInput")
    wvT = nc.dram_tensor("wvT", [D, D], b16, kind="ExternalInput")
    woT = nc.dram_tensor("woT", [D, D], b16, kind="ExternalInput")
    w1T = nc.dram_tensor("w1T", [D, DFF], b16, kind="ExternalInput")
    w2T = nc.dram_tensor("w2T", [DFF, D4], b16, kind="ExternalInput")
    w3T = nc.dram_tensor("w3T", [D4, D], b16, kind="ExternalInput")
    bcols = nc.dram_tensor("bcols", [C, NBCOL], f32, kind="ExternalInput")
    bv_ = nc.dram_tensor("bv_", [1, D], f32, kind="ExternalInput")
    outT = nc.dram_tensor("outT", [D, TOK], f32, kind="ExternalOutput")

    with tile.TileContext(nc) as tc:
        P = tc.alloc_tile_pool(name="main", bufs=1)
        psum = tc.alloc_tile_pool(name="psum", bufs=8, space="PSUM")

        def mk(shape, dtype, tag):
            return P.tile(shape, dtype, tag=tag, name=tag)

        ones = mk([C, 1], b16, "ones")
        nc.vector.memset(ones, 1.0)
        eps1 = mk([1, 1], f32, "eps1")
        nc.vector.memset(eps1, EPS)

        sb_bc = mk([C, NBCOL], f32, "bcols")
        nc.sync.dma_start(out=sb_bc, in_=bcols.ap())

        def col(name, c):
            s, n = _BCOLS[name]
            assert c < n
            return sb_bc[:, s + c:s + c + 1]

        sb_bvb = []
        for oh in range(NTH):
            t = mk([C, TH], f32, f"bvb{oh}")
            src = bv_.ap()[0:1, oh * TH:(oh + 1) * TH]
            nc.sync.dma_start(out=t, in_=src.partition_broadcast(C))
            sb_bvb.append(t)

        # ================= per-token-half pipeline =================
        for th in range(NTH):
            tsl = slice(th * TH, (th + 1) * TH)

            # --- load x (bf16 for matmuls, fp32 for residual) ---
            # --- V = x @ Wv.T + bv (token-major [tok, feat]) ---
            xb, wv_t = [], []
            for k in range(KD):
                t = mk([C, TH], b16, f"xb{k}")
                nc.sync.dma_start(out=t, in_=xTb.ap()[k * C:(k + 1) * C, tsl])
                xb.append(t)
                t = mk([C, D], b16, f"swA{k}")
                nc.sync.dma_start(out=t, in_=wvT.ap()[k * C:(k + 1) * C, :])
                wv_t.append(t)
            sv = [[None] * NTH for _ in range(BPT)]
            for tb in range(BPT):
                for oh in range(NTH):
                    pv = psum.tile([C, TH], f32, tag="ps",
                                   name=f"psv{tb}{oh}{th}")
                    for k in range(KD):
                        nc.tensor.matmul(
                            pv, xb[k][:, tb * C:(tb + 1) * C],
                            wv_t[k][:, oh * TH:(oh + 1) * TH],
                            start=(k == 0), stop=(k == KD - 1))
                    t = mk([C, TH], b16, f"sv{tb}_{oh}")
                    nc.vector.tensor_add(t, pv, sb_bvb[oh])
                    sv[tb][oh] = t

            # --- Q, K for all heads (feature-major [dk, tok]) ---
            wq_t, wk_t = [], []
            for k in range(KD):
                t = mk([C, D], b16, f"swA{8 + k}")
                nc.sync.dma_start(out=t, in_=wqT.ap()[k * C:(k + 1) * C, :])
                wq_t.append(t)
            for k in range(KD):
                t = mk([C, D], b16, f"swA{k}")  # reuse wv slots
                nc.sync.dma_start(out=t, in_=wkT.ap()[k * C:(k + 1) * C, :])
                wk_t.append(t)
            qh, kh = [None] * H, [None] * H

            def emit_qk(h):
                pq = psum.tile([C, TH], f32, tag="ps", name=f"psq{h}{th}")
                for k in range(KD):
                    nc.tensor.matmul(pq, wq_t[k][:, h * C:(h + 1) * C],
                                     xb[k], start=(k == 0), stop=(k == KD - 1))
                qh[h] = mk([C, TH], b16, f"q{h}")
                nc.scalar.activation(qh[h], pq, Act.Identity,
                                     bias=col("bqs", h), scale=SCALE)
                pk = psum.tile([C, TH], f32, tag="ps", name=f"psk{h}{th}")
                for k in range(KD):
                    nc.tensor.matmul(pk, wk_t[k][:, h * C:(h + 1) * C],
                                     xb[k], start=(k == 0), stop=(k == KD - 1))
                kh[h] = mk([C, TH], b16, f"k{h}")
                nc.scalar.activation(kh[h], pk, Act.Identity,
                                     bias=col("bk", h))

            # --- attention (shift-free softmax, no transposes), with QK
            # chains interleaved in program order as PE filler ---
            ctx = [None] * KD
            for h in range(KD):
                ctx[h] = mk([C, TH], b16, f"ctx{h}")
            for h in range(3):
                emit_qk(h)
            for h in range(H):
                if h + 3 < H:
                    emit_qk(h + 3)
                hsl = [slice(tb * C, (tb + 1) * C) for tb in range(BPT)]
                vsls = [sv[tb][h // BPT][:, (h % BPT) * C:(h % BPT + 1) * C]
                        for tb in range(BPT)]
                # scoresT[k,q] for all 4 batches packed into one bank
                pbs = psum.tile([C, TH], f32, tag="ps", name=f"pbs{h}{th}")
                for tb in range(BPT):
                    nc.tensor.matmul(pbs[:, hsl[tb]], kh[h][:, hsl[tb]],
                                     qh[h][:, hsl[tb]], start=True, stop=True)
                e = mk([C, TH], b16, f"eA{h % 4}")
                nc.scalar.activation(e, pbs, Act.Exp)
                # per-column sums of exp via one wide PE ones-reduce
                prow = psum.tile([1, TH], f32, tag="ps", name=f"prow{h}{th}")
                nc.tensor.matmul(prow, ones, e, start=True, stop=True)
                rrow = mk([1, TH], f32, f"rro{h % 4}")
                nc.vector.reciprocal(rrow, prow)
                # avT_unnorm per batch, packed into one bank
                pbav = psum.tile([C, TH], f32, tag="ps", name=f"pbav{h}{th}")
                for tb in range(BPT):
                    nc.tensor.matmul(pbav[:, hsl[tb]], vsls[tb],
                                     e[:, hsl[tb]], start=True, stop=True)
                avb = mk([C, TH], b16, f"avA{h % 4}")
                nc.scalar.copy(avb, pbav)
                # ctxT_unnorm per batch
                pbcx = psum.tile([C, TH], f32, tag="ps", name=f"pbcx{h}{th}")
                for tb in range(BPT):
                    nc.tensor.matmul(pbcx[:, hsl[tb]], vsls[tb],
                                     avb[:, hsl[tb]], start=True, stop=True)
                cxu = mk([C, TH], b16, f"ctxu{h % 4}")
                nc.scalar.copy(cxu, pbcx)
                # normalize the head's ctx columns by 1/rowsum
                rall = mk([C, TH], f32, f"rall{h % 4}")
                nc.gpsimd.partition_broadcast(rall, rrow)
                nc.vector.tensor_mul(ctx[h], cxu, rall)

            # --- attn_out = ctx @ Wo.T + bo ; r1 = x + attn_out ---
            wo_t = []
            for k in range(KD):
                t = mk([C, D], b16, f"swA{8 + k}")  # reuse wq slots
                nc.sync.dma_start(out=t, in_=woT.ap()[k * C:(k + 1) * C, :])
                wo_t.append(t)
            xf = []
            for k in range(KD):
                t = mk([C, TH], f32, f"xf{k}")
                nc.sync.dma_start(out=t, in_=xT.ap()[k * C:(k + 1) * C, tsl])
                xf.append(t)
            r1 = [None] * KD
            for c in range(KD):
                po = psum.tile([C, TH], f32, tag="ps", name=f"pso{c}{th}")
                for k in range(KD):
                    nc.tensor.matmul(po, wo_t[k][:, c * C:(c + 1) * C],
                                     ctx[k], start=(k == 0), stop=(k == KD - 1))
                t = mk([C, TH], f32, f"r1h{c}")
                nc.vector.scalar_tensor_tensor(t, po, col("bo", c), xf[c],
                                               op0=Alu.add, op1=Alu.add)
                r1[c] = t

            # --- layernorm helper (over features = partitions) ---
            def layer_norm(src, wname, bname, out_mk, also_b16):
                vb, sq = [], []
                for c in range(KD):
                    tvb = mk([C, TH], b16, f"svb{c}")
                    nc.vector.tensor_copy(tvb, src[c])
                    vb.append(tvb)
                    tsq = mk([C, TH], b16, f"ssq{c}")
                    nc.scalar.activation(tsq, src[c], Act.Square)
                    sq.append(tsq)
                ps1 = psum.tile([1, TH], f32, tag="ps", name=f"ps1{th}")
                for c in range(KD):
                    nc.tensor.matmul(ps1, ones, vb[c], start=(c == 0),
                                     stop=(c == KD - 1))
                ps2 = psum.tile([1, TH], f32, tag="ps", name=f"ps2{th}")
                for c in range(KD):
                    nc.tensor.matmul(ps2, ones, sq[c], start=(c == 0),
                                     stop=(c == KD - 1))
                mu = mk([1, TH], f32, "rowA")
                nc.scalar.mul(mu, ps1, 1.0 / D)
                msq = mk([1, TH], f32, "rowB")
                nc.scalar.mul(msq, ps2, 1.0 / D)
                mu2 = mk([1, TH], f32, "rowC")
                nc.vector.tensor_mul(mu2, mu, mu)
                var = mk([1, TH], f32, "rowD")
                nc.vector.tensor_sub(var, msq, mu2)
                sd = mk([1, TH], f32, "rowC")
                nc.scalar.activation(sd, var, Act.Sqrt, bias=eps1)
                rinv = mk([1, TH], f32, "rowB")
                nc.vector.reciprocal(rinv, sd)
                mub = mk([C, TH], f32, "mub")
                nc.gpsimd.partition_broadcast(mub, mu)
                rb = mk([C, TH], f32, "rb")
                nc.gpsimd.partition_broadcast(rb, rinv)
                outs_f, outs_b = [], []
                for c in range(KD):
                    t1 = mk([C, TH], f32, "t1")
                    nc.vector.tensor_sub(t1, src[c], mub)
                    t2 = mk([C, TH], f32, f"t2{c % 2}")
                    nc.vector.tensor_mul(t2, t1, rb)
                    to = out_mk(c)
                    nc.scalar.activation(to, t2, Act.Identity,
                                         bias=col(bname, c),
                                         scale=col(wname, c))
                    outs_f.append(to)
                    if also_b16:
                        tb16 = mk([C, TH], b16, f"sv{c // 2}_{c % 2}")
                        nc.vector.tensor_copy(tb16, to)
                        outs_b.append(tb16)
                return outs_f, outs_b

            # --- LN1: h fp32 (r1 slots) + bf16 copy (sv slots) ---
            hf, hb = layer_norm(r1, "l1w", "l1b",
                                lambda c: mk([C, TH], f32, f"r1h{c}"), True)

            # --- f1 = relu(h @ W1.T + b1); W1 streamed via GpSimd DGE ---
            f1tag = [f"ctx{i}" for i in range(8)] + [f"q{i}" for i in range(8)]
            f1 = [None] * KFF
            for blk in range(KFF // 4):
                pf = [psum.tile([C, TH], f32, tag="ps",
                                name=f"psf1{blk}{i}{th}") for i in range(4)]
                for k in range(KD):
                    ws = mk([C, 4 * C], b16, f"ws{(blk * KD + k) % 12}")
                    eng = nc.gpsimd if k % 2 == 0 else nc.scalar
                    eng.dma_start(
                        out=ws, in_=w1T.ap()[k * C:(k + 1) * C,
                                             blk * 4 * C:(blk + 1) * 4 * C])
                    for i in range(4):
                        nc.tensor.matmul(pf[i], ws[:, i * C:(i + 1) * C],
                                         hb[k], start=(k == 0),
                                         stop=(k == KD - 1))
                for i in range(4):
                    c = blk * 4 + i
                    t = mk([C, TH], b16, f1tag[c])
                    nc.scalar.activation(t, pf[i], Act.Relu, bias=col("b1", c))
                    f1[c] = t

            # --- f2 = relu(f1 @ W2.T + b2) ---
            f2tag = [f"xf{i}" for i in range(8)] + \
                    [f"svb{i}" for i in range(8)] + \
                    [f"ssq{i}" for i in range(8)] + \
                    [f"sf2_{i}" for i in range(8)]
            f2 = [None] * K4
            for blk in range(K4 // 4):
                pf = [psum.tile([C, TH], f32, tag="ps",
                                name=f"psf2{blk}{i}{th}") for i in range(4)]
                for k in range(KFF):
                    ws = mk([C, 4 * C], b16, f"ws{(blk * KFF + k) % 12}")
                    eng = nc.gpsimd if k % 2 == 0 else nc.scalar
                    eng.dma_start(
                        out=ws, in_=w2T.ap()[k * C:(k + 1) * C,
                                             blk * 4 * C:(blk + 1) * 4 * C])
                    for i in range(4):
                        nc.tensor.matmul(pf[i], ws[:, i * C:(i + 1) * C],
                                         f1[k], start=(k == 0),
                                         stop=(k == KFF - 1))
                for i in range(4):
                    c = blk * 4 + i
                    t = mk([C, TH], b16, f2tag[c])
                    nc.scalar.activation(t, pf[i], Act.Relu, bias=col("b2", c))
                    f2[c] = t

            # --- f3 = f2 @ W3.T + b3 ; r2 = h + f3 ---
            r2 = [None] * KD
            for blk in range(KD // 4):
                pf = [psum.tile([C, TH], f32, tag="ps",
                                name=f"psf3{blk}{i}{th}") for i in range(4)]
                for k in range(K4):
                    ws = mk([C, 4 * C], b16, f"ws{(blk * K4 + k) % 12}")
                    eng = nc.gpsimd if k % 2 == 0 else nc.scalar
                    eng.dma_start(
                        out=ws, in_=w3T.ap()[k * C:(k + 1) * C,
                                             blk * 4 * C:(blk + 1) * 4 * C])
                    for i in range(4):
                        nc.tensor.matmul(pf[i], ws[:, i * C:(i + 1) * C],
                                         f2[k], start=(k == 0),
                                         stop=(k == K4 - 1))
                for i in range(4):
                    c = blk * 4 + i
                    t = mk([C, TH], f32, f"r2o{c}")
                    nc.vector.scalar_tensor_tensor(t, pf[i], col("b3", c),
                                                   hf[c], op0=Alu.add,
                                                   op1=Alu.add)
                    r2[c] = t

            # --- LN2 -> out, DMA ---
            of, _ = layer_norm(r2, "l2w", "l2b",
                               lambda c: mk([C, TH], f32, f"r2o{c}"), False)
            for c in range(KD):
                nc.sync.dma_start(out=outT.ap()[c * C:(c + 1) * C, tsl],
                                  in_=of[c])

        psum.release()
        P.release()

    nc.compile()
    return nc


def _get_nc():
    if "nc" not in _nc_cache:
        _nc_cache["nc"] = _build()
    return _nc_cache["nc"]


def kernel(x, mask, Wq, bq, Wk, bk, Wv, bv, Wo, bo, ln1_w, ln1_b,
           W1, b1, W2, b2, W3, b3, ln2_w, ln2_b):
    from concourse.bass_utils import run_bass_kernel_spmd

    bf = ml_dtypes.bfloat16
    f32 = np.float32

    assert np.all(np.asarray(mask) != 0), \
        "kernel specialized for the all-ones mask this module is run with"

    x = np.asarray(x, f32)

    def chunks(v, n):
        v = np.asarray(v, f32).reshape(n, C)
        return [v[i] for i in range(n)]

    cols = []
    for (nm, (_st, cnt)), src in zip(
            _BCOLS.items(),
            [np.asarray(bq, f32) * SCALE, bk, bo, b1, b2, b3,
             ln1_w, ln1_b, ln2_w, ln2_b]):
        cols += chunks(src, cnt)
    bcols = np.stack(cols, axis=1).astype(f32)  # [C, NBCOL]

    shared = {
        "wqT": np.ascontiguousarray(np.asarray(Wq, f32).T.astype(bf)),
        "wkT": np.ascontiguousarray(np.asarray(Wk, f32).T.astype(bf)),
        "wvT": np.ascontiguousarray(np.asarray(Wv, f32).T.astype(bf)),
        "woT": np.ascontiguousarray(np.asarray(Wo, f32).T.astype(bf)),
        "w1T": np.ascontiguousarray(np.asarray(W1, f32).T.astype(bf)),
        "w2T": np.ascontiguousarray(np.asarray(W2, f32).T.astype(bf)),
        "w3T": np.ascontiguousarray(np.asarray(W3, f32).T.astype(bf)),
        "bcols": bcols,
        "bv_": np.asarray(bv, f32).reshape(1, D),
    }

    in_maps = []
    for c in range(NCORES):
        xc = np.ascontiguousarray(
            x[c * BPC:(c + 1) * BPC].reshape(TOK, D).T)
        m = dict(shared)
        m["xT"] = xc
        m["xTb"] = xc.astype(bf)
        in_maps.append(m)

    nc = _get_nc()
    res = run_bass_kernel_spmd(nc, in_maps, core_ids=list(range(NCORES)),
                               trace=_trace["on"])
    _trace["res"] = res

    out = np.empty((B, S, D), f32)
    for c in range(NCORES):
        out[c * BPC:(c + 1) * BPC] = np.asarray(
            res.results[c]["outT"]).T.reshape(BPC, S, D)
    return out


# revision 27
# speedup vs baseline: 1.0579x; 1.0133x over previous
"""Fused decoder-layer kernel for Trainium2 (8 NeuronCores, data-parallel over batch).

Self-contained: hardcodes shapes B=64, S=128, D=1024, H=8, DK=128, DFF=2048.

Strategy:
  - Shard batch 8-ways (8 batches = 1024 tokens per core). No collectives.
  - Activations kept feature-major ("transposed", [feat, tok]) on device so
    every matmul Y = X @ W.T becomes Y.T with the contraction dim on SBUF
    partitions; weights are pre-transposed on host and cast bf16.
  - The layer is processed in two token-halves of 512 (4 batches each);
    attention and layernorm are token-local, so the halves are independent
    pipelines that share SBUF slots (Tile inserts the waits).
  - Attention without transposes: scoresT[k,q] = K_h^T-free matmul, exp on
    ScalarE (shift-free softmax: logits are O(1) for this module), both
    V-matmuls use token-major V as the stationary operand, and the softmax
    normalization (per-q column scale) is applied at the final drain using a
    GpSimd partition-all-reduce of exp(scores). One PSUM bank per unit
    (scores/av/ctx packed as column slices) keeps many units in flight.
  - LayerNorm over features (= partitions) via PE ones-reduction for
    sum / sum-of-squares, GpSimd partition_broadcast for mean/invstd rows.
  - Residuals and layernorm in fp32; matmul operands bf16 (fp32 PSUM accum).
  - Streamed FFN weights are DMA'd from the GpSimd DGE (the Sync DGE is a
    serial ~0.6us-per-descriptor bottleneck); everything else from Sync.
"""

import sys

if "/opt/trn_rl_repo" not in sys.path:
    sys.path.insert(0, "/opt/trn_rl_repo")

import numpy as np
import ml_dtypes

B, S, D, H = 64, 128, 1024, 8
DK = D // H
DFF = 2048
D4 = 4 * D
EPS = 1e-12
NCORES = 8
BPC = B // NCORES          # batches per core
TOK = BPC * S              # tokens per core = 1024
TH = 512                   # token half processed per pipeline pass
NTH = TOK // TH            # 2
BPT = TH // S              # batches per half = 4
C = 128                    # chunk / partition size
KD = D // C                # 8
KFF = DFF // C             # 16
K4 = D4 // C               # 32
SCALE = 1.0 / np.sqrt(DK)

# packed bias/ln columns: name -> (start, count)
_BCOLS = {}
_off = 0
for _nm, _cnt in [("bqs", KD), ("bk", KD), ("bo", KD), ("b1", KFF),
                  ("b2", K4), ("b3", KD), ("l1w", KD), ("l1b", KD),
                  ("l2w", KD), ("l2b", KD)]:
    _BCOLS[_nm] = (_off, _cnt)
    _off += _cnt
NBCOL = _off

_nc_cache = {}
_trace = {"on": False, "res": None}


def _build():
    import concourse.bass as bass
    import concourse.mybir as mybir
    import concourse.tile as tile
    import concourse.bass_isa as bass_isa
    from concourse import bacc

    f32 = mybir.dt.float32
    b16 = mybir.dt.bfloat16
    Alu = mybir.AluOpType
    Act = mybir.ActivationFunctionType
    AX = mybir.AxisListType

    nc = bacc.Bacc("TRN2", target_bir_lowering=False, debug=False,
                   num_devices=NCORES)

    # ---- DRAM I/O ------------------------------------------------------
    xT = nc.dram_tensor("xT", [D, TOK], f32, kind="ExternalInput")
    xTb = nc.dram_tensor("xTb", [D, TOK], b16, kind="ExternalInput")
    wqT = nc.dram_tensor("wqT", [D, D], b16, kind="ExternalInput")
    wkT = nc.dram_tensor("wkT", [D, D], b16, kind="ExternalInput")
    wvT = nc.dram_tensor("wvT", [D, D], b16, kind="ExternalInput")
    woT = nc.dram_tensor("woT", [D, D], b16, kind="ExternalInput")
    w1T = nc.dram_tensor("w1T", [D, DFF], b16, kind="ExternalInput")
    w2T = nc.dram_tensor("w2T", [DFF, D4], b16, kind="ExternalInput")
    w3T = nc.dram_tensor("w3T", [D4, D], b16, kind="ExternalInput")
    bcols = nc.dram_tensor("bcols", [C, NBCOL], f32, kind="ExternalInput")
    bv_ = nc.dram_tensor("bv_", [1, D], f32, kind="ExternalInput")
    outT = nc.dram_tensor("outT", [D, TOK], f32, kind="ExternalOutput")

    with tile.TileContext(nc) as tc:
        P = tc.alloc_tile_pool(name="main", bufs=1)
        psum = tc.alloc_tile_pool(name="psum", bufs=8, space="PSUM")

        def mk(shape, dtype, tag):
            return P.tile(shape, dtype, tag=tag, name=tag)

        ones = mk([C, 1], b16, "ones")
        nc.vector.memset(ones, 1.0)
        eps1 = mk([1, 1], f32, "eps1")
        nc.vector.memset(eps1, EPS)

        sb_bc = mk([C, NBCOL], f32, "bcols")
        nc.sync.dma_start(out=sb_bc, in_=bcols.ap())

        def col(name, c):
            s, n = _BCOLS[name]
            assert c < n
            return sb_bc[:, s + c:s + c + 1]

        sb_bvb = []
        for oh in range(NTH):
            t = mk([C, TH], f32, f"bvb{oh}")
            src = bv_.ap()[0:1, oh * TH:(oh + 1) * TH]
            nc.sync.dma_start(out=t, in_=src.partition_broadcast(C))
            sb_bvb.append(t)

        # ================= per-token-half pipeline =================
        for th in range(NTH):
            tsl = slice(th * TH, (th + 1) * TH)

            # --- load x (bf16 for matmuls, fp32 for residual) ---
            # --- V = x @ Wv.T + bv (token-major [tok, feat]) ---
            xb, wv_t = [], []
            for k in range(KD):
                t = mk([C, TH], b16, f"xb{k}")
                nc.sync.dma_start(out=t, in_=xTb.ap()[k * C:(k + 1) * C, tsl])
                xb.append(t)
                t = mk([C, D], b16, f"swA{k}")
                nc.sync.dma_start(out=t, in_=wvT.ap()[k * C:(k + 1) * C, :])
                wv_t.append(t)
            sv = [[None] * NTH for _ in range(BPT)]
            for tb in range(BPT):
                for oh in range(NTH):
                    pv = psum.tile([C, TH], f32, tag="ps",
                                   name=f"psv{tb}{oh}{th}")
                    for k in range(KD):
                        nc.tensor.matmul(
                            pv, xb[k][:, tb * C:(tb + 1) * C],
                            wv_t[k][:, oh * TH:(oh + 1) * TH],
                            start=(k == 0), stop=(k == KD - 1))
                    t = mk([C, TH], b16, f"sv{tb}_{oh}")
                    nc.vector.tensor_add(t, pv, sb_bvb[oh])
                    sv[tb][oh] = t

            # --- Q, K for all heads (feature-major [dk, tok]) ---
            wq_t, wk_t = [], []
            for k in range(KD):
                t = mk([C, D], b16, f"swA{8 + k}")
                nc.sync.dma_start(out=t, in_=wqT.ap()[k * C:(k + 1) * C, :])
                wq_t.append(t)
            for k in range(KD):
                t = mk([C, D], b16, f"swA{k}")  # reuse wv slots
                nc.sync.dma_start(out=t, in_=wkT.ap()[k * C:(k + 1) * C, :])
                wk_t.append(t)
            qh, kh = [None] * H, [None] * H

            def emit_qk(h):
                pq = psum.tile([C, TH], f32, tag="ps", name=f"psq{h}{th}")
                for k in range(KD):
                    nc.tensor.matmul(pq, wq_t[k][:, h * C:(h + 1) * C],
                                     xb[k], start=(k == 0), stop=(k == KD - 1))
                qh[h] = mk([C, TH], b16, f"q{h}")
                nc.scalar.activation(qh[h], pq, Act.Identity,
                                     bias=col("bqs", h), scale=SCALE)
                pk = psum.tile([C, TH], f32, tag="ps", name=f"psk{h}{th}")
                for k in range(KD):
                    nc.tensor.matmul(pk, wk_t[k][:, h * C:(h + 1) * C],
                                     xb[k], start=(k == 0), stop=(k == KD - 1))
                kh[h] = mk([C, TH], b16, f"k{h}")
                nc.scalar.activation(kh[h], pk, Act.Identity,
                                     bias=col("bk", h))

            # --- attention (shift-free softmax, no transposes), with QK
            # chains interleaved in program order as PE filler ---
            ctx = [None] * KD
            for h in range(KD):
                ctx[h] = mk([C, TH], b16, f"ctx{h}")
            for h in range(3):
                emit_qk(h)
            wo_t = [[None] * KD for _ in range(2)]
            for h in range(H):
                if h + 3 < H:
                    emit_qk(h + 3)
                hsl = [slice(tb * C, (tb + 1) * C) for tb in range(BPT)]
                vsls = [sv[tb][h // BPT][:, (h % BPT) * C:(h % BPT + 1) * C]
                        for tb in range(BPT)]
                # scoresT[k,q] for all 4 batches packed into one bank
                pbs = psum.tile([C, TH], f32, tag="ps", name=f"pbs{h}{th}")
                for tb in range(BPT):
                    nc.tensor.matmul(pbs[:, hsl[tb]], kh[h][:, hsl[tb]],
                                     qh[h][:, hsl[tb]], start=True, stop=True)
                e = mk([C, TH], b16, f"eA{h % 4}")
                nc.scalar.activation(e, pbs, Act.Exp)
                # per-column sums of exp via one wide PE ones-reduce
                prow = psum.tile([1, TH], f32, tag="ps", name=f"prow{h}{th}")
                nc.tensor.matmul(prow, ones, e, start=True, stop=True)
                rrow = mk([1, TH], f32, f"rro{h % 4}")
                nc.vector.reciprocal(rrow, prow)
                # avT_unnorm per batch, packed into one bank
                pbav = psum.tile([C, TH], f32, tag="ps", name=f"pbav{h}{th}")
                for tb in range(BPT):
                    nc.tensor.matmul(pbav[:, hsl[tb]], vsls[tb],
                                     e[:, hsl[tb]], start=True, stop=True)
                avb = mk([C, TH], b16, f"avA{h % 4}")
                nc.scalar.copy(avb, pbav)
                # ctxT_unnorm per batch
                pbcx = psum.tile([C, TH], f32, tag="ps", name=f"pbcx{h}{th}")
                for tb in range(BPT):
                    nc.tensor.matmul(pbcx[:, hsl[tb]], vsls[tb],
                                     avb[:, hsl[tb]], start=True, stop=True)
                cxu = mk([C, TH], b16, f"ctxu{h % 4}")
                nc.scalar.copy(cxu, pbcx)
                # normalize the head's ctx columns by 1/rowsum
                rall = mk([C, TH], f32, f"rall{h % 4}")
                nc.gpsimd.partition_broadcast(rall, rrow)
                nc.vector.tensor_mul(ctx[h], cxu, rall)
                # stream Wo halves into the q/k slots this head just freed
                t = mk([C, TH], b16, f"q{h}")
                nc.sync.dma_start(out=t, in_=woT.ap()[h * C:(h + 1) * C, 0:TH])
                wo_t[0][h] = t
                t = mk([C, TH], b16, f"k{h}")
                nc.sync.dma_start(out=t, in_=woT.ap()[h * C:(h + 1) * C,
                                                      TH:D])
                wo_t[1][h] = t

            # --- attn_out = ctx @ Wo.T + bo ; r1 = x + attn_out ---
            xf = []
            for k in range(KD):
                t = mk([C, TH], f32, f"xf{k}")
                nc.sync.dma_start(out=t, in_=xT.ap()[k * C:(k + 1) * C, tsl])
                xf.append(t)
            r1 = [None] * KD
            for c in range(KD):
                po = psum.tile([C, TH], f32, tag="ps", name=f"pso{c}{th}")
                for k in range(KD):
                    nc.tensor.matmul(
                        po, wo_t[c // 4][k][:, (c % 4) * C:(c % 4 + 1) * C],
                        ctx[k], start=(k == 0), stop=(k == KD - 1))
                t = mk([C, TH], f32, f"r1h{c}")
                nc.vector.scalar_tensor_tensor(t, po, col("bo", c), xf[c],
                                               op0=Alu.add, op1=Alu.add)
                r1[c] = t

            # --- layernorm helper (over features = partitions) ---
            def layer_norm(src, wname, bname, out_mk, also_b16):
                vb, sq = [], []
                for c in range(KD):
                    tvb = mk([C, TH], b16, f"svb{c}")
                    nc.vector.tensor_copy(tvb, src[c])
                    vb.append(tvb)
                    tsq = mk([C, TH], b16, f"ssq{c}")
                    nc.scalar.activation(tsq, src[c], Act.Square)
                    sq.append(tsq)
                ps1 = psum.tile([1, TH], f32, tag="ps", name=f"ps1{th}")
                for c in range(KD):
                    nc.tensor.matmul(ps1, ones, vb[c], start=(c == 0),
                                     stop=(c == KD - 1))
                ps2 = psum.tile([1, TH], f32, tag="ps", name=f"ps2{th}")
                for c in range(KD):
                    nc.tensor.matmul(ps2, ones, sq[c], start=(c == 0),
                                     stop=(c == KD - 1))
                mu = mk([1, TH], f32, "rowA")
                nc.scalar.mul(mu, ps1, 1.0 / D)
                msq = mk([1, TH], f32, "rowB")
                nc.scalar.mul(msq, ps2, 1.0 / D)
                mu2 = mk([1, TH], f32, "rowC")
                nc.vector.tensor_mul(mu2, mu, mu)
                var = mk([1, TH], f32, "rowD")
                nc.vector.tensor_sub(var, msq, mu2)
                rinv = mk([1, TH], f32, "rowB")
                nc.scalar.activation(rinv, var, Act.Abs_reciprocal_sqrt,
                                     bias=eps1)
                mub = mk([C, TH], f32, "mub")
                nc.gpsimd.partition_broadcast(mub, mu)
                rb = mk([C, TH], f32, "rb")
                nc.gpsimd.partition_broadcast(rb, rinv)
                outs_f, outs_b = [], []
                for c in range(KD):
                    t1 = mk([C, TH], f32, "t1")
                    nc.vector.tensor_sub(t1, src[c], mub)
                    t2 = mk([C, TH], f32, f"t2{c % 2}")
                    nc.vector.tensor_mul(t2, t1, rb)
                    to = out_mk(c)
                    nc.scalar.activation(to, t2, Act.Identity,
                                         bias=col(bname, c),
                                         scale=col(wname, c))
                    outs_f.append(to)
                    if also_b16:
                        tb16 = mk([C, TH], b16, f"sv{c // 2}_{c % 2}")
                        nc.vector.tensor_copy(tb16, to)
                        outs_b.append(tb16)
                return outs_f, outs_b

            # --- LN1: h fp32 (r1 slots) + bf16 copy (sv slots) ---
            hf, hb = layer_norm(r1, "l1w", "l1b",
                                lambda c: mk([C, TH], f32, f"r1h{c}"), True)

            # --- f1 = relu(h @ W1.T + b1); W1 streamed via GpSimd DGE ---
            f1tag = [f"ctx{i}" for i in range(8)] + [f"q{i}" for i in range(8)]
            f1 = [None] * KFF
            for blk in range(KFF // 4):
                pf = [psum.tile([C, TH], f32, tag="ps",
                                name=f"psf1{blk}{i}{th}") for i in range(4)]
                for k in range(KD):
                    ws = mk([C, 4 * C], b16, f"ws{(blk * KD + k) % 12}")
                    eng = nc.gpsimd if k % 2 == 0 else nc.scalar
                    eng.dma_start(
                        out=ws, in_=w1T.ap()[k * C:(k + 1) * C,
                                             blk * 4 * C:(blk + 1) * 4 * C])
                    for i in range(4):
                        nc.tensor.matmul(pf[i], ws[:, i * C:(i + 1) * C],
                                         hb[k], start=(k == 0),
                                         stop=(k == KD - 1))
                for i in range(4):
                    c = blk * 4 + i
                    t = mk([C, TH], b16, f1tag[c])
                    nc.scalar.activation(t, pf[i], Act.Relu, bias=col("b1", c))
                    f1[c] = t

            # --- f2 = relu(f1 @ W2.T + b2) ---
            f2tag = [f"xf{i}" for i in range(8)] + \
                    [f"svb{i}" for i in range(8)] + \
                    [f"ssq{i}" for i in range(8)] + \
                    [f"sf2_{i}" for i in range(8)]
            f2 = [None] * K4
            for blk in range(K4 // 4):
                pf = [psum.tile([C, TH], f32, tag="ps",
                                name=f"psf2{blk}{i}{th}") for i in range(4)]
                for k in range(KFF):
                    ws = mk([C, 4 * C], b16, f"ws{(blk * KFF + k) % 12}")
                    eng = nc.gpsimd if k % 2 == 0 else nc.scalar
                    eng.dma_start(
                        out=ws, in_=w2T.ap()[k * C:(k + 1) * C,
                                             blk * 4 * C:(blk + 1) * 4 * C])
                    for i in range(4):
                        nc.tensor.matmul(pf[i], ws[:, i * C:(i + 1) * C],
                                         f1[k], start=(k == 0),
                                         stop=(k == KFF - 1))
                for i in range(4):
                    c = blk * 4 + i
                    t = mk([C, TH], b16, f2tag[c])
                    nc.scalar.activation(t, pf[i], Act.Relu, bias=col("b2", c))
                    f2[c] = t

            # --- f3 = f2 @ W3.T + b3 ; r2 = h + f3 ---
            r2 = [None] * KD
            for blk in range(KD // 2):
                pf = [psum.tile([C, TH], f32, tag="ps",
                                name=f"psf3{blk}{i}{th}") for i in range(2)]
                for k in range(K4):
                    ws = mk([C, 2 * C], b16, f"ws{(blk * K4 + k) % 12}")
                    eng = nc.gpsimd if k % 2 == 0 else nc.scalar
                    eng.dma_start(
                        out=ws, in_=w3T.ap()[k * C:(k + 1) * C,
                                             blk * 2 * C:(blk + 1) * 2 * C])
                    for i in range(2):
                        nc.tensor.matmul(pf[i], ws[:, i * C:(i + 1) * C],
                                         f2[k], start=(k == 0),
                                         stop=(k == K4 - 1))
                for i in range(2):
                    c = blk * 2 + i
                    t = mk([C, TH], f32, f"r2o{c}")
                    nc.vector.scalar_tensor_tensor(t, pf[i], col("b3", c),
                                                   hf[c], op0=Alu.add,
                                                   op1=Alu.add)
                    r2[c] = t

            # --- LN2 -> out, DMA ---
            of, _ = layer_norm(r2, "l2w", "l2b",
                               lambda c: mk([C, TH], f32, f"r2o{c}"), False)
            for c in range(KD):
                nc.sync.dma_start(out=outT.ap()[c * C:(c + 1) * C, tsl],
                                  in_=of[c])

        psum.release()
        P.release()

    nc.compile()
    return nc


def _get_nc():
    if "nc" not in _nc_cache:
        _nc_cache["nc"] = _build()
    return _nc_cache["nc"]


def kernel(x, mask, Wq, bq, Wk, bk, Wv, bv, Wo, bo, ln1_w, ln1_b,
           W1, b1, W2, b2, W3, b3, ln2_w, ln2_b):
    from concourse.bass_utils import run_bass_kernel_spmd

    bf = ml_dtypes.bfloat16
    f32 = np.float32

    assert np.all(np.asarray(mask) != 0), \
        "kernel specialized for the all-ones mask this module is run with"

    x = np.asarray(x, f32)

    def chunks(v, n):
        v = np.asarray(v, f32).reshape(n, C)
        return [v[i] for i in range(n)]

    cols = []
    for (nm, (_st, cnt)), src in zip(
            _BCOLS.items(),
            [np.asarray(bq, f32) * SCALE, bk, bo, b1, b2, b3,
             ln1_w, ln1_b, ln2_w, ln2_b]):
        cols += chunks(src, cnt)
    bcols = np.stack(cols, axis=1).astype(f32)  # [C, NBCOL]

    shared = {
        "wqT": np.ascontiguousarray(np.asarray(Wq, f32).T.astype(bf)),
        "wkT": np.ascontiguousarray(np.asarray(Wk, f32).T.astype(bf)),
        "wvT": np.ascontiguousarray(np.asarray(Wv, f32).T.astype(bf)),
        "woT": np.ascontiguousarray(np.asarray(Wo, f32).T.astype(bf)),
        "w1T": np.ascontiguousarray(np.asarray(W1, f32).T.astype(bf)),
        "w2T": np.ascontiguousarray(np.asarray(W2, f32).T.astype(bf)),
        "w3T": np.ascontiguousarray(np.asarray(W3, f32).T.astype(bf)),
        "bcols": bcols,
        "bv_": np.asarray(bv, f32).reshape(1, D),
    }

    in_maps = []
    for c in range(NCORES):
        xc = np.ascontiguousarray(
            x[c * BPC:(c + 1) * BPC].reshape(TOK, D).T)
        m = dict(shared)
        m["xT"] = xc
        m["xTb"] = xc.astype(bf)
        in_maps.append(m)

    nc = _get_nc()
    res = run_bass_kernel_spmd(nc, in_maps, core_ids=list(range(NCORES)),
                               trace=_trace["on"])
    _trace["res"] = res

    out = np.empty((B, S, D), f32)
    for c in range(NCORES):
        out[c * BPC:(c + 1) * BPC] = np.asarray(
            res.results[c]["outT"]).T.reshape(BPC, S, D)
    return out
